# revision 9
# baseline (speedup 1.0000x reference)
"""Trainium2 Bass kernel for nn_MultiHeadCrossAttention (fp8 DoubleRow scores).

Sharding: 8 cores = 4 batches x 2 head-groups (8 local heads each).
Per-core pipeline:
  - K/Q projections emitted in a "dr" layout: partitions = 4 heads x 32 dims
    per head-quad, free slot i in {0,1} holds dim halves d = i*32 + j. Host
    pre-scales Wq/Wk (and biases) by 4 so the fp8e4 cast uses more range.
  - Partial rotary (dims 0..31 = slot 0) via 32-lane stream_shuffle blend,
    cast to fp8e4 on write; pass dims (slot 1) cast directly from PSUM.
  - scores^T per head via fp8 DoubleRow matmuls [32,2,128] x [32,2,512]
    (2x column rate; the two slots reassemble the 64-dim contraction).
  - exp on ACT over [128,1024] PSUM tiles (scale=0.125/16), double-buffered.
  - attn@V in bf16 with an appended ones column giving softmax denominators.
  - Normalize via reciprocal + DMA partition-broadcast; out-projection is
    row-split; host sums the two head-group partials and adds the bias.
Emission interleaves scores/exp/attn at klt granularity with a filler queue
(v-projection, second-half q-projection, out-projection) to keep the PE fed
while ACT (the binding engine, ~290us of exp) stays saturated.
"""

import sys

sys.path.insert(0, "/opt/trn_rl_repo")

from collections import deque

import numpy as np
import ml_dtypes
from contextlib import ExitStack

import concourse.bass as bass
import concourse.bacc as bacc
import concourse.mybir as mybir
from concourse.tile import TileContext

DIM = 1024
H = 16
HD = 64
ROT = 32
B = 4
QL = 2048
KL = 2048
G = 2                # head-group (tensor-parallel) factor
HL = H // G          # 8 local heads
DL = HL * HD         # 512 local feature dims
NPAIR = HL // 2
NCORE = 8
SC = 4.0             # fp8 pre-scale on Wq/Wk (and their biases)
SCL = 0.125 / (SC * SC)   # exp scale: 1/sqrt(HD) / SC^2

F32 = mybir.dt.float32
F8 = mybir.dt.float8e4
BF16 = mybir.dt.bfloat16
AFT = mybir.ActivationFunctionType
ALU = mybir.AluOpType
DRMODE = mybir.MatmulPerfMode.DoubleRow
bf16 = ml_dtypes.bfloat16
f8e4 = ml_dtypes.float8_e4m3

_NC_CACHE = {}

# dr-layout column permutation: dr col (mt*128 + hl*32 + j) <- local W row
_ROWMAP = np.zeros(DL, np.int64)
for _mt in range(4):
    _quad, _i = _mt // 2, _mt % 2
    for _hl in range(4):
        for _j in range(32):
            _ROWMAP[_mt * 128 + _hl * 32 + _j] = \
                (_quad * 4 + _hl) * 64 + _i * 32 + _j


def _rot_patterns_dr():
    """ccd/ssd [128, QL]: rotary blend patterns for the dr slot-0 layout.

    Partition p = hl*32 + j (same for every head): ccd[p,t] = cos(t*invf[j//2]),
    ssd[p,t] = +/- sin(...), sign - for even j, + for odd j (interleaved
    rotate_half with partner j^1 supplied by stream_shuffle).
    """
    inv_freq = 1.0 / (10000.0 ** (np.arange(0, ROT, 2, dtype=np.float64) / ROT))
    t = np.arange(QL, dtype=np.float64)
    freqs = t[None, :] * inv_freq[:, None]          # [16, QL]
    cc32 = np.zeros((32, QL), np.float64)
    ss32 = np.zeros((32, QL), np.float64)
    for j in range(32):
        cc32[j] = np.cos(freqs[j // 2])
        ss32[j] = np.sin(freqs[j // 2]) * (-1.0 if j % 2 == 0 else 1.0)
    ccd = np.tile(cc32, (4, 1)).astype(np.float32)  # [128, QL]
    ssd = np.tile(ss32, (4, 1)).astype(np.float32)
    return ccd, ssd


def _build_nc():
    if "nc" in _NC_CACHE:
        return _NC_CACHE["nc"]
    nc = bacc.Bacc("TRN2", target_bir_lowering=False)

    d = {}
    for name, shape, dt in [
        ("qT", [DIM, QL], BF16), ("kT", [DIM, KL], BF16), ("vT", [DIM, KL], BF16),
        ("wqT", [DIM, DL], BF16), ("wkT", [DIM, DL], BF16), ("wvT", [DIM, DL], BF16),
        ("woT", [DL, DIM], BF16),
        ("bqp", [128, 4], F32), ("bkp", [128, 4], F32),
        ("bv", [1, DL], BF16), ("ones1", [1, 128], BF16),
        ("ccd", [128, QL], BF16), ("ssd", [128, QL], BF16),
    ]:
        d[name] = nc.dram_tensor(name, shape, dt, kind="ExternalInput")
    out_d = nc.dram_tensor("out", [QL, DIM], F32, kind="ExternalOutput")

    qT_t = d["qT"].rearrange("(a p) n -> a p n", p=128)     # [8, 128, QL]
    kT_t = d["kT"].rearrange("(a p) n -> a p n", p=128)
    vT_t = d["vT"].rearrange("(a p) n -> a p n", p=128)
    wqT_t = d["wqT"].rearrange("(a p) n -> a p n", p=128)   # [8, 128, DL]
    wkT_t = d["wkT"].rearrange("(a p) n -> a p n", p=128)
    wvT_t = d["wvT"].rearrange("(a p) n -> a p n", p=128)
    woT_t = d["woT"].rearrange("(a p) n -> a p n", p=128)   # [4, 128, DIM]
    out_t = out_d.rearrange("(a p) n -> a p n", p=128)      # [16, 128, DIM]

    SWAP_MASK = [(j + 1 if j % 2 == 0 else j - 1) for j in range(32)]

    with TileContext(nc) as tc, ExitStack() as top:
        consts = top.enter_context(tc.tile_pool(name="consts", bufs=1))
        bq_s = consts.tile([128, 4], F32)
        nc.scalar.dma_start(out=bq_s, in_=d["bqp"][:, :])
        bk_s = consts.tile([128, 4], F32)
        nc.scalar.dma_start(out=bk_s, in_=d["bkp"][:, :])
        bv_s = consts.tile([1, DL], BF16)
        nc.scalar.dma_start(out=bv_s, in_=d["bv"][:, :])
        ones_s = consts.tile([1, 128], BF16)
        nc.scalar.dma_start(out=ones_s, in_=d["ones1"][:, :])
        ccd_s = consts.tile([128, QL], BF16)
        nc.scalar.dma_start(out=ccd_s, in_=d["ccd"][:, :])
        ssd_s = consts.tile([128, QL], BF16)
        nc.scalar.dma_start(out=ssd_s, in_=d["ssd"][:, :])
        wo_s = [consts.tile([128, DIM], BF16, tag=f"wo{i}", name=f"wo{i}")
                for i in range(NPAIR)]
        for i in range(NPAIR):
            nc.scalar.dma_start(out=wo_s[i], in_=woT_t[i])
        # Warm the ACT exp table early (hides the ~1.3us table load).
        warm = consts.tile([1, 8], F32)
        nc.scalar.activation(out=warm, in_=ones_s[0:1, 0:8], func=AFT.Exp)

        # ---- persistent activations ----
        # kdr/qdr: [128, 2*T] fp8, free layout slot-major (slot i at i*T);
        # matmuls read them through a (p, 2, T) split view.
        pers = top.enter_context(tc.tile_pool(name="pers", bufs=1))
        kdr = [pers.tile([128, 2 * KL], F8, tag=f"kdr{g}", name=f"kdr{g}")
               for g in range(2)]
        qdr = [pers.tile([128, 2 * QL], F8, tag=f"qdr{g}", name=f"qdr{g}")
               for g in range(2)]
        kdr_v = [t.rearrange("p (i n) -> p i n", i=2) for t in kdr]
        qdr_v = [t.rearrange("p (i n) -> p i n", i=2) for t in qdr]
        vh_pool = top.enter_context(tc.tile_pool(name="vh", bufs=16))
        vh = [vh_pool.tile([128, NPAIR * 130], BF16, tag="vh", name=f"vh{i}")
              for i in range(16)]
        at_pool = top.enter_context(tc.tile_pool(name="atn", bufs=NPAIR))
        apT = [at_pool.tile([128, QL], BF16, tag="at", name=f"apT{i}")
               for i in range(NPAIR)]

        with ExitStack() as ph:
            stage = ph.enter_context(tc.tile_pool(name="stage", bufs=12))
            vstage = ph.enter_context(tc.tile_pool(name="vstage", bufs=8))
            wkq_p = ph.enter_context(tc.tile_pool(name="wkq", bufs=8))
            wv_p = ph.enter_context(tc.tile_pool(name="wvp", bufs=8))
            tmp_p = ph.enter_context(tc.tile_pool(name="tmp", bufs=2))
            sw_p = ph.enter_context(tc.tile_pool(name="swp", bufs=2))
            t2_p = ph.enter_context(tc.tile_pool(name="t2p", bufs=2))
            ets_p = ph.enter_context(tc.tile_pool(name="ets", bufs=5))
            atu_p = ph.enter_context(tc.tile_pool(name="atu", bufs=2))
            rc_p = ph.enter_context(tc.tile_pool(name="rcp", bufs=2))
            bt_p = ph.enter_context(tc.tile_pool(name="btp", bufs=2))
            out_p = ph.enter_context(tc.tile_pool(name="outp", bufs=2))
            dscr = ph.enter_context(tc.tile_pool(name="dscr", bufs=8, space="DRAM"))
            psS = ph.enter_context(tc.tile_pool(name="psS", bufs=2, space="PSUM"))
            psF = ph.enter_context(tc.tile_pool(name="psF", bufs=1, space="PSUM"))
            psPA = ph.enter_context(tc.tile_pool(name="psPA", bufs=1, space="PSUM"))

            # ---- input staging DMAs ----
            ks = [stage.tile([128, KL], BF16, tag="st", name=f"ks{a}")
                  for a in range(8)]
            for a in range(8):
                eng = nc.sync if a % 2 == 0 else nc.gpsimd
                eng.dma_start(out=ks[a], in_=kT_t[a])
            wks = [wkq_p.tile([128, DL], BF16, tag="w", name=f"wks{a}")
                   for a in range(8)]
            for a in range(8):
                nc.scalar.dma_start(out=wks[a], in_=wkT_t[a])
            vs = [vstage.tile([128, KL], BF16, tag="vst", name=f"vs{a}")
                  for a in range(8)]
            for a in range(8):
                nc.gpsimd.dma_start(out=vs[a], in_=vT_t[a])
            wvs = [wv_p.tile([128, DL], BF16, tag="wv", name=f"wvs{a}")
                   for a in range(8)]
            for a in range(8):
                nc.scalar.dma_start(out=wvs[a], in_=wvT_t[a])
            qs = [stage.tile([128, QL], BF16, tag="st", name=f"qs{a}")
                  for a in range(8)]
            for a in range(8):
                nc.sync.dma_start(out=qs[a], in_=qT_t[a])
            wqs = [wkq_p.tile([128, DL], BF16, tag="w", name=f"wqs{a}")
                   for a in range(8)]
            for a in range(8):
                nc.scalar.dma_start(out=wqs[a], in_=wqT_t[a])

            # ---- projection helpers ----
            ps_rot = []  # small rotation of PSUM tiles for lead-in chains

            def lead_ps(name):
                # rotate psF -> psS -> psPA during lead-in to keep PE fed
                idx = len(ps_rot) % 4
                pool, tag = ((psF, "F"), (psS, "S"), (psS, "S"),
                             (psPA, "PA"))[idx]
                ps_rot.append(None)
                return pool.tile([128, 1024], F32, tag=tag, name=name)

            def proj_half(xs, ws, mt, c, name):
                """One c2-half chain of a K/Q projection pair-tile -> PSUM."""
                ps = lead_ps(name)
                for a in range(8):
                    for n in range(2):
                        nc.tensor.matmul(
                            ps[:, n * 512:(n + 1) * 512],
                            lhsT=ws[a][:, mt * 128:(mt + 1) * 128],
                            rhs=xs[a][:, c * 1024 + n * 512:
                                      c * 1024 + (n + 1) * 512],
                            start=(a == 0), stop=(a == 7),
                        )
                return ps

            def rot_write(dst, ps, b_s, mt, c):
                """Rotary blend of slot-0 dims: PSUM half -> fp8 dst slot 0."""
                cs = slice(c * 1024, (c + 1) * 1024)
                ds = slice(c * 1024, (c + 1) * 1024)  # slot 0 at offset 0
                qt = tmp_p.tile([128, 1024], BF16, tag="tmp")
                nc.vector.tensor_scalar_add(out=qt, in0=ps,
                                            scalar1=b_s[:, mt:mt + 1])
                sw = sw_p.tile([128, 1024], BF16, tag="sw")
                nc.vector.stream_shuffle(out=sw, in_=qt, mask=SWAP_MASK)
                nc.vector.tensor_tensor(out=sw, in0=sw, in1=ssd_s[:, cs],
                                        op=ALU.mult)
                t2 = t2_p.tile([128, 1024], BF16, tag="t2")
                nc.vector.tensor_tensor(out=t2, in0=qt, in1=ccd_s[:, cs],
                                        op=ALU.mult)
                nc.vector.tensor_tensor(out=dst[:, ds], in0=sw, in1=t2,
                                        op=ALU.add)

            def pass_write(dst, ps, b_s, mt, c):
                """Pass dims: PSUM half + bias -> fp8 dst slot 1 directly."""
                ds = slice(KL + c * 1024, KL + (c + 1) * 1024)
                nc.vector.tensor_scalar_add(out=dst[:, ds], in0=ps,
                                            scalar1=b_s[:, mt:mt + 1])

            def kq_quad(xs, ws, b_s, dst_dr, quad):
                """Project + rotary one head-quad into dst_dr (fp8)."""
                mt_rot, mt_pass = quad * 2, quad * 2 + 1
                for c in range(2):
                    ps = proj_half(xs, ws, mt_rot, c, f"pr{quad}{c}")
                    rot_write(dst_dr, ps, b_s, mt_rot, c)
                for c in range(2):
                    ps = proj_half(xs, ws, mt_pass, c, f"pp{quad}{c}")
                    pass_write(dst_dr, ps, b_s, mt_pass, c)

            # ---- k projection (both quads) -> kdr ----
            kq_quad(ks, wks, bk_s, kdr[0], 0)
            kq_quad(ks, wks, bk_s, kdr[1], 1)
            # ---- q projection quad 0 -> qdr[0] (quad 1 deferred to fillers)
            kq_quad(qs, wqs, bq_s, qdr[0], 0)

            # ---- filler queue ----
            fillers = deque()

            def pull(n=1):
                for _ in range(n):
                    if fillers:
                        fillers.popleft()()

            def vproj_closure(t):
                def go():
                    ps = psF.tile([128, 512], F32, tag="F", name=f"vp{t}")
                    for a in range(8):
                        nc.tensor.matmul(
                            ps,
                            lhsT=vs[a][:, t * 128:(t + 1) * 128],
                            rhs=wvs[a],
                            start=(a == 0), stop=False,
                        )
                    nc.tensor.matmul(ps, lhsT=ones_s, rhs=bv_s,
                                     start=False, stop=True)
                    vtr = vh[t].rearrange("p (g h e) -> p g h e", h=2, e=65)
                    nc.vector.memset(vtr[:, :, :, 64:65], 1.0)
                    psr = ps.rearrange("p (g h e) -> p g h e", h=2, e=64)
                    nc.vector.tensor_copy(out=vtr[:, :, :, 0:64], in_=psr)
                return go

            fillers.extend(vproj_closure(t) for t in range(16))

            def qproj_q1_closures():
                """quad-1 q projection split into small chunks."""
                out = []
                state = {}

                def mm_chunk(mt, c, a0, name):
                    def go():
                        if a0 == 0:
                            state[(mt, c)] = psF.tile([128, 1024], F32,
                                                      tag="F", name=name)
                        ps = state[(mt, c)]
                        for a in range(a0, a0 + 4):
                            for n in range(2):
                                nc.tensor.matmul(
                                    ps[:, n * 512:(n + 1) * 512],
                                    lhsT=wqs[a][:, mt * 128:(mt + 1) * 128],
                                    rhs=qs[a][:, c * 1024 + n * 512:
                                              c * 1024 + (n + 1) * 512],
                                    start=(a == 0), stop=(a == 7),
                                )
                    return go

                for c in range(2):
                    out.append(mm_chunk(2, c, 0, f"q1r{c}"))
                    out.append(mm_chunk(2, c, 4, f"q1r{c}"))
                    out.append(lambda c=c: rot_write(qdr[1], state[(2, c)],
                                                     bq_s, 2, c))
                for c in range(2):
                    out.append(mm_chunk(3, c, 0, f"q1p{c}"))
                    out.append(mm_chunk(3, c, 4, f"q1p{c}"))
                    out.append(lambda c=c: pass_write(qdr[1], state[(3, c)],
                                                      bq_s, 3, c))
                return out

            fillers.extend(qproj_q1_closures())

            def outproj_closures(qts):
                out = []
                state = {}

                def chain(qt, dc):
                    def go():
                        if dc == 0:
                            state[qt] = psF.tile([128, 1024], F32, tag="F",
                                                 name=f"op{qt}")
                        ps = state[qt]
                        for p in range(NPAIR):
                            nc.tensor.matmul(
                                ps[:, dc * 512:(dc + 1) * 512],
                                lhsT=apT[p][:, qt * 128:(qt + 1) * 128],
                                rhs=wo_s[p][:, dc * 512:(dc + 1) * 512],
                                start=(p == 0), stop=(p == NPAIR - 1),
                            )
                    return go

                def flush(qt):
                    def go():
                        ot = out_p.tile([128, DIM], F32, tag="o")
                        nc.vector.tensor_copy(out=ot, in_=state[qt])
                        nc.sync.dma_start(out=out_t[qt], in_=ot)
                    return go

                for qt in qts:
                    out.append(chain(qt, 0))
                    out.append(chain(qt, 1))
                    out.append(flush(qt))
                return out

            # ---- normalize: extract denominators, reciprocal, broadcast ----
            def normalize(pa, h, qc):
                p, hh = h // 2, h % 2
                atu = atu_p.tile([128, 1024], F32, tag="atu")
                nc.vector.tensor_copy(out=atu[0:65, :], in_=pa[0:65, :])
                ds = dscr.tile([1, 1024], F32, tag="dsc")
                nc.sync.dma_start(out=ds, in_=atu[64:65, :])
                rc8 = rc_p.tile([128, 8], F32, tag="rc8")
                nc.sync.dma_start(out=rc8,
                                  in_=ds.rearrange("a (p e) -> (a p) e", p=128))
                rc8b = rc_p.tile([128, 8], BF16, tag="rc8b")
                with nc.allow_low_precision(
                        reason="per-query softmax denominators; bf16 "
                               "reciprocal adds ~0.4% which is within budget"):
                    nc.vector.reciprocal(out=rc8b, in_=rc8)
                ds2 = dscr.tile([1, 1024], BF16, tag="ds2")
                nc.sync.dma_start(
                    out=ds2.rearrange("a (p e) -> (a p) e", p=128), in_=rc8b)
                bt = bt_p.tile([64, 1024], BF16, tag="bc")
                nc.sync.dma_start(out=bt,
                                  in_=ds2[0:1, :].to_broadcast([64, 1024]))
                nc.vector.tensor_tensor(
                    out=apT[p][hh * 64:(hh + 1) * 64,
                               qc * 1024:(qc + 1) * 1024],
                    in0=atu[0:64, :], in1=bt[0:64, :], op=ALU.mult)

            # ---- one attention unit: head h, query chunk qc (1024 cols) ----
            LAG = 2

            def unit(h, qc):
                quad, prow = h // 4, (h % 4) * 32
                p, hh = h // 2, h % 2
                pa = psPA.tile([128, 1024], F32, tag="PA", name=f"pa{h}{qc}")
                ets = {}

                def attn_step(t):
                    e = ets.pop(t)
                    lhs = vh[t][:, p * 130 + hh * 65: p * 130 + (hh + 1) * 65]
                    for n in range(2):
                        nc.tensor.matmul(
                            pa[0:65, n * 512:(n + 1) * 512],
                            lhsT=lhs,
                            rhs=e[:, n * 512:(n + 1) * 512],
                            start=(t == 0), stop=(t == 15),
                        )

                for t in range(16):
                    ps = psS.tile([128, 1024], F32, tag="S", name=f"s{h}{qc}{t}")
                    for n in range(2):
                        nc.tensor.matmul(
                            ps[:, n * 512:(n + 1) * 512],
                            lhsT=kdr_v[quad][prow:prow + 32, :,
                                             t * 128:(t + 1) * 128],
                            rhs=qdr_v[quad][prow:prow + 32, :,
                                            qc * 1024 + n * 512:
                                            qc * 1024 + (n + 1) * 512],
                            start=True, stop=True, perf_mode=DRMODE,
                            tile_position=(prow, 0),
                        )
                    e = ets_p.tile([128, 1024], BF16, tag="ets")
                    nc.scalar.activation(out=e, in_=ps, func=AFT.Exp,
                                         scale=SCL)
                    ets[t] = e
                    if t >= LAG:
                        attn_step(t - LAG)
                    pull(1)
                for t in range(16 - LAG, 16):
                    attn_step(t)
                normalize(pa, h, qc)

            # ---- attention phases ----
            for h in range(8):
                unit(h, 0)
            fillers.extend(outproj_closures(range(8)))
            for h in range(8):
                unit(h, 1)
            # drain remaining fillers (outproj qt 0..7 leftovers)
            while fillers:
                pull(1)
            for fn in outproj_closures(range(8, 16)):
                fn()

    nc.compile()
    _NC_CACHE["nc"] = nc
    return nc


def _make_in_maps(q, k, v, Wq, bq, Wk, bk, Wv, bv, Wo, bo):
    q, k, v = (np.asarray(x, np.float32) for x in (q, k, v))
    Wq, Wk, Wv, Wo = (np.asarray(x, np.float32) for x in (Wq, Wk, Wv, Wo))
    bq, bk, bv, bo = (np.asarray(x, np.float32) for x in (bq, bk, bv, bo))
    ccd, ssd = _rot_patterns_dr()
    ones1 = np.ones((1, 128), np.float32)
    in_maps = []
    for c in range(NCORE):
        b, g = divmod(c, G)
        gs = slice(g * DL, (g + 1) * DL)
        wq_g, bq_g = Wq[gs, :], bq[gs]
        wk_g, bk_g = Wk[gs, :], bk[gs]
        # dr layout: permute projection output dims, pre-scale by SC
        wq_dr = (wq_g[_ROWMAP, :] * SC).T            # [1024, 512]
        wk_dr = (wk_g[_ROWMAP, :] * SC).T
        bq_dr = (bq_g[_ROWMAP] * SC).reshape(4, 128).T   # [128, 4]
        bk_dr = (bk_g[_ROWMAP] * SC).reshape(4, 128).T
        in_maps.append({
            "qT": np.ascontiguousarray(q[b].T).astype(bf16),
            "kT": np.ascontiguousarray(k[b].T).astype(bf16),
            "vT": np.ascontiguousarray(v[b].T).astype(bf16),
            "wqT": np.ascontiguousarray(wq_dr).astype(bf16),
            "wkT": np.ascontiguousarray(wk_dr).astype(bf16),
            "wvT": np.ascontiguousarray(Wv[gs, :].T).astype(bf16),
            "woT": np.ascontiguousarray(Wo[:, gs].T).astype(bf16),
            "bqp": np.ascontiguousarray(bq_dr),
            "bkp": np.ascontiguousarray(bk_dr),
            "bv": np.ascontiguousarray(bv[gs][None, :]).astype(bf16),
            "ones1": ones1.astype(bf16),
            "ccd": ccd.astype(bf16), "ssd": ssd.astype(bf16),
        })
    return in_maps


def run(inputs: dict, trace: bool = False, tmpdir: str | None = None):
    """Returns (out [B, QL, DIM] f32, exec_time_ns or None)."""
    from concourse.bass_utils import run_bass_kernel_spmd

    nc = _build_nc()
    in_maps = _make_in_maps(**inputs)
    res = run_bass_kernel_spmd(nc, in_maps, list(range(NCORE)), trace=trace,
                               tmpdir=tmpdir)
    globals()["LAST_RES"] = res
    bo = np.asarray(inputs["bo"], np.float32)
    outs = [res.results[i]["out"] for i in range(NCORE)]
    out = np.stack([outs[G * b] + outs[G * b + 1] for b in range(B)])
    out += bo[None, None, :]
    return out.astype(np.float32), res.exec_time_ns


def kernel(**inputs) -> np.ndarray:
    out, _ = run(inputs, trace=False)
    return out


# revision 12
# speedup vs baseline: 1.1951x; 1.1951x over previous
"""Trainium2 Bass kernel for nn_MultiHeadCrossAttention (fp8 DoubleRow scores).

Sharding: 8 cores = 4 batches x 2 head-groups (8 local heads each).
Per-core pipeline:
  - K/Q projections in a "dr" layout: partitions = 4 heads x 32 dims per
    head-quad, free slot i in {0,1} holds dim halves d = i*32 + j. Host
    pre-scales Wq/Wk and their biases by 4 so fp8e4 sees a good range.
  - Partial rotary (dims 0..31 = slot 0) via 32-lane stream_shuffle blend,
    cast to fp8e4 on write; pass dims (slot 1) cast directly from PSUM.
  - scores^T per head via fp8 DoubleRow matmuls [32,2,128] x [32,2,512]
    (half cycles per column; the two slots reassemble the 64-dim dot).
  - exp on ACT over [128,1024] PSUM tiles (scale=0.125/16), double-buffered.
  - attn@V in bf16 with an appended ones column giving softmax denominators.
  - Normalize via reciprocal + DMA partition-broadcast; out-projection is
    row-split; host sums the two head-group partials and adds the bias.
Emission is a 256-tick score stream (2 DR matmuls + 1 exp per tick) with the
attention stream trailing by LAG ticks and a filler deque (v-projection,
second-half q-projection, out-projection) dosed one quantum per tick, so the
PE pipeline never queues an instruction whose inputs are not long since ready
while ACT (the binding engine, ~290us of exp) stays saturated.
"""

import sys

sys.path.insert(0, "/opt/trn_rl_repo")

from collections import deque

import numpy as np
import ml_dtypes
from contextlib import ExitStack

import concourse.bass as bass
import concourse.bacc as bacc
import concourse.mybir as mybir
from concourse.tile import TileContext

DIM = 1024
H = 16
HD = 64
ROT = 32
B = 4
QL = 2048
KL = 2048
G = 2                # head-group (tensor-parallel) factor
HL = H // G          # 8 local heads
DL = HL * HD         # 512 local feature dims
NPAIR = HL // 2
NCORE = 8
SC = 4.0             # fp8 pre-scale on Wq/Wk (and their biases)
SCL = 0.125 / (SC * SC)   # exp scale: 1/sqrt(HD) / SC^2
LAG = 8              # attn stream lag behind the score stream, in klt ticks

F32 = mybir.dt.float32
F8 = mybir.dt.float8e4
BF16 = mybir.dt.bfloat16
AFT = mybir.ActivationFunctionType
ALU = mybir.AluOpType
DRMODE = mybir.MatmulPerfMode.DoubleRow
bf16 = ml_dtypes.bfloat16

_NC_CACHE = {}

# dr-layout column permutation: dr col (mt*128 + hl*32 + j) <- local W row
_ROWMAP = np.zeros(DL, np.int64)
for _mt in range(4):
    _quad, _i = _mt // 2, _mt % 2
    for _hl in range(4):
        for _j in range(32):
            _ROWMAP[_mt * 128 + _hl * 32 + _j] = \
                (_quad * 4 + _hl) * 64 + _i * 32 + _j


def _rot_patterns_dr():
    """ccd/ssd [128, QL]: rotary blend patterns for the dr slot-0 layout."""
    inv_freq = 1.0 / (10000.0 ** (np.arange(0, ROT, 2, dtype=np.float64) / ROT))
    t = np.arange(QL, dtype=np.float64)
    freqs = t[None, :] * inv_freq[:, None]          # [16, QL]
    cc32 = np.zeros((32, QL), np.float64)
    ss32 = np.zeros((32, QL), np.float64)
    for j in range(32):
        cc32[j] = np.cos(freqs[j // 2])
        ss32[j] = np.sin(freqs[j // 2]) * (-1.0 if j % 2 == 0 else 1.0)
    ccd = np.tile(cc32, (4, 1)).astype(np.float32)  # [128, QL]
    ssd = np.tile(ss32, (4, 1)).astype(np.float32)
    return ccd, ssd


def _build_nc():
    if "nc" in _NC_CACHE:
        return _NC_CACHE["nc"]
    nc = bacc.Bacc("TRN2", target_bir_lowering=False)

    d = {}
    for name, shape, dt in [
        ("qT", [DIM, QL], BF16), ("kT", [DIM, KL], BF16), ("vT", [DIM, KL], BF16),
        ("wqT", [DIM, DL], BF16), ("wkT", [DIM, DL], BF16), ("wvT", [DIM, DL], BF16),
        ("woT", [DL, DIM], BF16),
        ("bqp", [128, 4], F32), ("bkp", [128, 4], F32),
        ("bv", [1, DL], BF16), ("ones1", [1, 128], BF16),
        ("ccd", [128, QL], BF16), ("ssd", [128, QL], BF16),
    ]:
        d[name] = nc.dram_tensor(name, shape, dt, kind="ExternalInput")
    out_d = nc.dram_tensor("out", [QL, DIM], F32, kind="ExternalOutput")

    qT_t = d["qT"].rearrange("(a p) n -> a p n", p=128)     # [8, 128, QL]
    kT_t = d["kT"].rearrange("(a p) n -> a p n", p=128)
    vT_t = d["vT"].rearrange("(a p) n -> a p n", p=128)
    wqT_t = d["wqT"].rearrange("(a p) n -> a p n", p=128)   # [8, 128, DL]
    wkT_t = d["wkT"].rearrange("(a p) n -> a p n", p=128)
    wvT_t = d["wvT"].rearrange("(a p) n -> a p n", p=128)
    woT_t = d["woT"].rearrange("(a p) n -> a p n", p=128)   # [4, 128, DIM]
    out_t = out_d.rearrange("(a p) n -> a p n", p=128)      # [16, 128, DIM]

    SWAP_MASK = [(j + 1 if j % 2 == 0 else j - 1) for j in range(32)]

    with TileContext(nc) as tc, ExitStack() as top:
        consts = top.enter_context(tc.tile_pool(name="consts", bufs=1))
        bq_s = consts.tile([128, 4], F32)
        nc.scalar.dma_start(out=bq_s, in_=d["bqp"][:, :])
        bk_s = consts.tile([128, 4], F32)
        nc.scalar.dma_start(out=bk_s, in_=d["bkp"][:, :])
        bv_s = consts.tile([1, DL], BF16)
        nc.scalar.dma_start(out=bv_s, in_=d["bv"][:, :])
        ones_s = consts.tile([1, 128], BF16)
        nc.scalar.dma_start(out=ones_s, in_=d["ones1"][:, :])
        ccd_s = consts.tile([128, QL], BF16)
        nc.scalar.dma_start(out=ccd_s, in_=d["ccd"][:, :])
        ssd_s = consts.tile([128, QL], BF16)
        nc.scalar.dma_start(out=ssd_s, in_=d["ssd"][:, :])
        wo_s = [consts.tile([128, DIM], BF16, tag=f"wo{i}", name=f"wo{i}")
                for i in range(NPAIR)]
        for i in range(NPAIR):
            nc.scalar.dma_start(out=wo_s[i], in_=woT_t[i])
        warm = consts.tile([1, 8], F32)
        nc.scalar.activation(out=warm, in_=ones_s[0:1, 0:8], func=AFT.Exp)

        # ---- persistent activations ----
        pers = top.enter_context(tc.tile_pool(name="pers", bufs=1))
        kdr = [pers.tile([128, 2 * KL], F8, tag=f"kdr{g}", name=f"kdr{g}")
               for g in range(2)]
        qdr = [pers.tile([128, 2 * QL], F8, tag=f"qdr{g}", name=f"qdr{g}")
               for g in range(2)]
        kdr_v = [t.rearrange("p (i n) -> p i n", i=2) for t in kdr]
        qdr_v = [t.rearrange("p (i n) -> p i n", i=2) for t in qdr]
        vh_pool = top.enter_context(tc.tile_pool(name="vh", bufs=16))
        vh = [vh_pool.tile([128, NPAIR * 130], BF16, tag="vh", name=f"vh{i}")
              for i in range(16)]
        at_pool = top.enter_context(tc.tile_pool(name="atn", bufs=NPAIR))
        apT = [at_pool.tile([128, QL], BF16, tag="at", name=f"apT{i}")
               for i in range(NPAIR)]

        with ExitStack() as ph:
            stage = ph.enter_context(tc.tile_pool(name="stage", bufs=12))
            vstage = ph.enter_context(tc.tile_pool(name="vstage", bufs=8))
            wkq_p = ph.enter_context(tc.tile_pool(name="wkq", bufs=8))
            wv_p = ph.enter_context(tc.tile_pool(name="wvp", bufs=8))
            tmp_p = ph.enter_context(tc.tile_pool(name="tmp", bufs=2))
            sw_p = ph.enter_context(tc.tile_pool(name="swp", bufs=1))
            t2_p = ph.enter_context(tc.tile_pool(name="t2p", bufs=1))
            ets_p = ph.enter_context(tc.tile_pool(name="ets", bufs=LAG + 2))
            atu_p = ph.enter_context(tc.tile_pool(name="atu", bufs=2))
            rc_p = ph.enter_context(tc.tile_pool(name="rcp", bufs=2))
            bt_p = ph.enter_context(tc.tile_pool(name="btp", bufs=2))
            out_p = ph.enter_context(tc.tile_pool(name="outp", bufs=1))
            dscr = ph.enter_context(tc.tile_pool(name="dscr", bufs=8, space="DRAM"))
            psS = ph.enter_context(tc.tile_pool(name="psS", bufs=2, space="PSUM"))
            psF = ph.enter_context(tc.tile_pool(name="psF", bufs=1, space="PSUM"))
            psPA = ph.enter_context(tc.tile_pool(name="psPA", bufs=1, space="PSUM"))

            # ---- input staging DMAs ----
            ks = [stage.tile([128, KL], BF16, tag="st", name=f"ks{a}")
                  for a in range(8)]
            for a in range(8):
                eng = nc.sync if a % 2 == 0 else nc.gpsimd
                eng.dma_start(out=ks[a], in_=kT_t[a])
            wks = [wkq_p.tile([128, DL], BF16, tag="w", name=f"wks{a}")
                   for a in range(8)]
            for a in range(8):
                nc.scalar.dma_start(out=wks[a], in_=wkT_t[a])
            vs = [vstage.tile([128, KL], BF16, tag="vst", name=f"vs{a}")
                  for a in range(8)]
            for a in range(8):
                nc.gpsimd.dma_start(out=vs[a], in_=vT_t[a])
            wvs = [wv_p.tile([128, DL], BF16, tag="wv", name=f"wvs{a}")
                   for a in range(8)]
            for a in range(8):
                nc.scalar.dma_start(out=wvs[a], in_=wvT_t[a])
            qs = [stage.tile([128, QL], BF16, tag="st", name=f"qs{a}")
                  for a in range(8)]
            for a in range(8):
                nc.sync.dma_start(out=qs[a], in_=qT_t[a])
            wqs = [wkq_p.tile([128, DL], BF16, tag="w", name=f"wqs{a}")
                   for a in range(8)]
            for a in range(8):
                nc.scalar.dma_start(out=wqs[a], in_=wqT_t[a])

            # ---- projection plumbing ----
            ps_rot = []

            def lead_ps(name):
                # rotate PSUM pools during the lead-in to keep the PE fed
                idx = len(ps_rot) % 4
                pool, tag = ((psS, "S"), (psPA, "PA"), (psS, "S"),
                             (psF, "F"))[idx]
                ps_rot.append(None)
                return pool.tile([128, 1024], F32, tag=tag, name=name)

            def proj_part(xs, ws, mt, c, a0, a1, ps):
                for a in range(a0, a1):
                    for n in range(2):
                        nc.tensor.matmul(
                            ps[:, n * 512:(n + 1) * 512],
                            lhsT=ws[a][:, mt * 128:(mt + 1) * 128],
                            rhs=xs[a][:, c * 1024 + n * 512:
                                      c * 1024 + (n + 1) * 512],
                            start=(a == 0), stop=(a == 7),
                        )

            def rot_write(dst, ps, b_s, mt, c):
                """Rotary blend of slot-0 dims: PSUM half -> fp8 dst slot 0."""
                cs = slice(c * 1024, (c + 1) * 1024)
                qt = tmp_p.tile([128, 1024], BF16, tag="tmp")
                nc.vector.tensor_scalar_add(out=qt, in0=ps,
                                            scalar1=b_s[:, mt:mt + 1])
                sw = sw_p.tile([128, 1024], BF16, tag="sw")
                nc.vector.stream_shuffle(out=sw, in_=qt, mask=SWAP_MASK)
                nc.vector.tensor_tensor(out=sw, in0=sw, in1=ssd_s[:, cs],
                                        op=ALU.mult)
                t2 = t2_p.tile([128, 1024], BF16, tag="t2")
                nc.vector.tensor_tensor(out=t2, in0=qt, in1=ccd_s[:, cs],
                                        op=ALU.mult)
                nc.vector.tensor_tensor(out=dst[:, cs], in0=sw, in1=t2,
                                        op=ALU.add)

            def pass_write(dst, ps, b_s, mt, c):
                """Pass dims: PSUM half + bias -> fp8 dst slot 1 directly."""
                ds = slice(KL + c * 1024, KL + (c + 1) * 1024)
                nc.vector.tensor_scalar_add(out=dst[:, ds], in0=ps,
                                            scalar1=b_s[:, mt:mt + 1])

            # ---- lead-in: full k projection, then q projection quad 0 ----
            def kq_quad(xs, ws, b_s, dst, quad, pref):
                mt_rot, mt_pass = quad * 2, quad * 2 + 1
                for c in range(2):
                    ps = lead_ps(f"{pref}r{c}")
                    proj_part(xs, ws, mt_rot, c, 0, 8, ps)
                    rot_write(dst, ps, b_s, mt_rot, c)
                for c in range(2):
                    ps = lead_ps(f"{pref}p{c}")
                    proj_part(xs, ws, mt_pass, c, 0, 8, ps)
                    pass_write(dst, ps, b_s, mt_pass, c)

            kq_quad(ks, wks, bk_s, kdr[0], 0, "k0")
            kq_quad(ks, wks, bk_s, kdr[1], 1, "k1")
            kq_quad(qs, wqs, bq_s, qdr[0], 0, "q0")

            # ---- filler queue ----
            fillers = deque()

            def pull(n=1):
                for _ in range(n):
                    if fillers:
                        fillers.popleft()()

            def vproj_closure(t):
                def go():
                    ps = psF.tile([128, 512], F32, tag="F", name=f"vp{t}")
                    for a in range(8):
                        nc.tensor.matmul(
                            ps,
                            lhsT=vs[a][:, t * 128:(t + 1) * 128],
                            rhs=wvs[a],
                            start=(a == 0), stop=False,
                        )
                    nc.tensor.matmul(ps, lhsT=ones_s, rhs=bv_s,
                                     start=False, stop=True)
                    vtr = vh[t].rearrange("p (g h e) -> p g h e", h=2, e=65)
                    nc.vector.memset(vtr[:, :, :, 64:65], 1.0)
                    psr = ps.rearrange("p (g h e) -> p g h e", h=2, e=64)
                    nc.vector.tensor_copy(out=vtr[:, :, :, 0:64], in_=psr)
                return go

            fillers.extend(vproj_closure(t) for t in range(16))

            def qproj_q1_closures():
                out = []
                state = {}

                def chunk(mt, c, a0, a1, name):
                    def go():
                        if a0 == 0:
                            state[(mt, c)] = psF.tile([128, 1024], F32,
                                                      tag="F", name=name)
                        proj_part(qs, wqs, mt, c, a0, a1, state[(mt, c)])
                    return go

                for c in range(2):
                    out.append(chunk(2, c, 0, 4, f"q2r{c}"))
                    out.append(chunk(2, c, 4, 8, f"q2r{c}"))
                    out.append(lambda c=c: rot_write(qdr[1], state[(2, c)],
                                                     bq_s, 2, c))
                for c in range(2):
                    out.append(chunk(3, c, 0, 4, f"q3p{c}"))
                    out.append(chunk(3, c, 4, 8, f"q3p{c}"))
                    out.append(lambda c=c: pass_write(qdr[1], state[(3, c)],
                                                      bq_s, 3, c))
                return out

            fillers.extend(qproj_q1_closures())

            def outproj_closures(qts):
                out = []
                state = {}

                def chain(qt, dc):
                    def go():
                        if dc == 0:
                            state[qt] = psF.tile([128, 1024], F32, tag="F",
                                                 name=f"op{qt}")
                        ps = state[qt]
                        for p in range(NPAIR):
                            nc.tensor.matmul(
                                ps[:, dc * 512:(dc + 1) * 512],
                                lhsT=apT[p][:, qt * 128:(qt + 1) * 128],
                                rhs=wo_s[p][:, dc * 512:(dc + 1) * 512],
                                start=(p == 0), stop=(p == NPAIR - 1),
                            )
                    return go

                def flush(qt):
                    def go():
                        ot = out_p.tile([128, DIM], F32, tag="o")
                        nc.vector.tensor_copy(out=ot, in_=state[qt])
                        nc.sync.dma_start(out=out_t[qt], in_=ot)
                    return go

                for qt in qts:
                    out.append(chain(qt, 0))
                    out.append(chain(qt, 1))
                    out.append(flush(qt))
                return out

            def normalize(pa, h, qc):
                p, hh = h // 2, h % 2
                atu = atu_p.tile([128, 1024], F32, tag="atu")
                nc.vector.tensor_copy(out=atu[0:65, :], in_=pa[0:65, :])
                ds = dscr.tile([1, 1024], F32, tag="dsc")
                nc.sync.dma_start(out=ds, in_=atu[64:65, :])
                rc8 = rc_p.tile([128, 8], F32, tag="rc8")
                nc.sync.dma_start(out=rc8,
                                  in_=ds.rearrange("a (p e) -> (a p) e", p=128))
                rc8b = rc_p.tile([128, 8], BF16, tag="rc8b")
                with nc.allow_low_precision(
                        reason="softmax denominators; bf16 reciprocal adds "
                               "~0.4% which is within the error budget"):
                    nc.vector.reciprocal(out=rc8b, in_=rc8)
                ds2 = dscr.tile([1, 1024], BF16, tag="ds2")
                nc.sync.dma_start(
                    out=ds2.rearrange("a (p e) -> (a p) e", p=128), in_=rc8b)
                bt = bt_p.tile([64, 1024], BF16, tag="bc")
                nc.sync.dma_start(out=bt,
                                  in_=ds2[0:1, :].to_broadcast([64, 1024]))
                nc.vector.tensor_tensor(
                    out=apT[p][hh * 64:(hh + 1) * 64,
                               qc * 1024:(qc + 1) * 1024],
                    in0=atu[0:64, :], in1=bt[0:64, :], op=ALU.mult)

            # ---- score stream + lagged attn stream + fillers ----
            ets = {}
            pa_cur = [None]

            def s_tick(t):
                u, klt = divmod(t, 16)
                h, qc = u % 8, u // 8
                quad, prow = h // 4, (h % 4) * 32
                ps = psS.tile([128, 1024], F32, tag="S", name=f"s{t}")
                for n in range(2):
                    nc.tensor.matmul(
                        ps[:, n * 512:(n + 1) * 512],
                        lhsT=kdr_v[quad][prow:prow + 32, :,
                                         klt * 128:(klt + 1) * 128],
                        rhs=qdr_v[quad][prow:prow + 32, :,
                                        qc * 1024 + n * 512:
                                        qc * 1024 + (n + 1) * 512],
                        start=True, stop=True, perf_mode=DRMODE,
                        tile_position=(prow, 0),
                    )
                e = ets_p.tile([128, 1024], BF16, tag="ets")
                nc.scalar.activation(out=e, in_=ps, func=AFT.Exp, scale=SCL)
                ets[t] = e

            def a_tick(a):
                u, klt = divmod(a, 16)
                h, qc = u % 8, u // 8
                p, hh = h // 2, h % 2
                if klt == 0:
                    pa_cur[0] = psPA.tile([128, 1024], F32, tag="PA",
                                          name=f"pa{u}")
                pa = pa_cur[0]
                e = ets.pop(a)
                lhs = vh[klt][:, p * 130 + hh * 65: p * 130 + (hh + 1) * 65]
                for n in range(2):
                    nc.tensor.matmul(
                        pa[0:65, n * 512:(n + 1) * 512],
                        lhsT=lhs,
                        rhs=e[:, n * 512:(n + 1) * 512],
                        start=(klt == 0), stop=(klt == 15),
                    )
                if klt == 15:
                    normalize(pa, h, qc)
                    if u == 7:            # qc=0 attn complete
                        fillers.extend(outproj_closures(range(8)))

            for t in range(256):
                s_tick(t)
                if t >= LAG:
                    a_tick(t - LAG)
                pull(2 if t < 24 else 1)
            for a in range(256 - LAG, 256):
                a_tick(a)
                pull(1)
            while fillers:
                pull(1)
            for fn in outproj_closures(range(8, 16)):
                fn()

    nc.compile()
    _NC_CACHE["nc"] = nc
    return nc


def _make_in_maps(q, k, v, Wq, bq, Wk, bk, Wv, bv, Wo, bo):
    q, k, v = (np.asarray(x, np.float32) for x in (q, k, v))
    Wq, Wk, Wv, Wo = (np.asarray(x, np.float32) for x in (Wq, Wk, Wv, Wo))
    bq, bk, bv, bo = (np.asarray(x, np.float32) for x in (bq, bk, bv, bo))
    ccd, ssd = _rot_patterns_dr()
    ones1 = np.ones((1, 128), np.float32)
    in_maps = []
    for c in range(NCORE):
        b, g = divmod(c, G)
        gs = slice(g * DL, (g + 1) * DL)
        wq_dr = (Wq[gs, :][_ROWMAP, :] * SC).T            # [1024, 512]
        wk_dr = (Wk[gs, :][_ROWMAP, :] * SC).T
        bq_dr = (bq[gs][_ROWMAP] * SC).reshape(4, 128).T  # [128, 4]
        bk_dr = (bk[gs][_ROWMAP] * SC).reshape(4, 128).T
        in_maps.append({
            "qT": np.ascontiguousarray(q[b].T).astype(bf16),
            "kT": np.ascontiguousarray(k[b].T).astype(bf16),
            "vT": np.ascontiguousarray(v[b].T).astype(bf16),
            "wqT": np.ascontiguousarray(wq_dr).astype(bf16),
            "wkT": np.ascontiguousarray(wk_dr).astype(bf16),
            "wvT": np.ascontiguousarray(Wv[gs, :].T).astype(bf16),
            "woT": np.ascontiguousarray(Wo[:, gs].T).astype(bf16),
            "bqp": np.ascontiguousarray(bq_dr),
            "bkp": np.ascontiguousarray(bk_dr),
            "bv": np.ascontiguousarray(bv[gs][None, :]).astype(bf16),
            "ones1": ones1.astype(bf16),
            "ccd": ccd.astype(bf16), "ssd": ssd.astype(bf16),
        })
    return in_maps


def run(inputs: dict, trace: bool = False, tmpdir: str | None = None):
    """Returns (out [B, QL, DIM] f32, exec_time_ns or None)."""
    from concourse.bass_utils import run_bass_kernel_spmd

    nc = _build_nc()
    in_maps = _make_in_maps(**inputs)
    res = run_bass_kernel_spmd(nc, in_maps, list(range(NCORE)), trace=trace,
                               tmpdir=tmpdir)
    globals()["LAST_RES"] = res
    bo = np.asarray(inputs["bo"], np.float32)
    outs = [res.results[i]["out"] for i in range(NCORE)]
    out = np.stack([outs[G * b] + outs[G * b + 1] for b in range(B)])
    out += bo[None, None, :]
    return out.astype(np.float32), res.exec_time_ns


def kernel(**inputs) -> np.ndarray:
    out, _ = run(inputs, trace=False)
    return out


# revision 14
# speedup vs baseline: 1.5736x; 1.3167x over previous
"""Trainium2 Bass kernel for nn_MultiHeadCrossAttention.

Sharding: 8 cores = 4 batches x 2 head-groups (8 local heads each).
Per-core pipeline:
  - Q/K projections via fp8e4 DoubleRow matmuls (K=128 partitions x 2 slots
    per instruction = 2 contraction tiles, full 2.4GHz rate). Host pre-scales
    inputs x8 and weights x16; the 1/128 descale is folded into the rotary
    cos/sin blend patterns, biases are pre-scaled x128.
  - V projection and everything downstream in bf16 (precision budget:
    fp8 q/k projection alone costs ~1.25e-2 rel err, within the 2e-2 gate).
  - Partial rotary via 32-lane stream_shuffle blend (head-pair layout).
  - scores^T per head: K=64 bf16 matmuls quadrant-packed via tile_position,
    alternating heads so adjacent instructions overlap in the PE array.
  - exp on ACT over [128,1024] PSUM tiles (triple-buffered so PE idle clumps
    into long runs that hold the PE at its ramped clock).
  - attn@V in bf16 with an appended ones column giving softmax denominators;
    normalize via reciprocal + DMA partition-broadcast; out-projection is
    row-split; host sums the two head-group partials and adds the bias.
Emission: lead-in (staging DMAs, fp8 K/Q projections + rotary, bf16 V
projection), then a 256-tick score stream (2 score matmuls + 1 exp per tick)
with the attention stream trailing by LAG ticks; out-projection chains are
dosed at unit boundaries so ACT stays saturated end-to-end.
"""

import sys

sys.path.insert(0, "/opt/trn_rl_repo")

from collections import deque

import numpy as np
import ml_dtypes
from contextlib import ExitStack

import concourse.bass as bass
import concourse.bacc as bacc
import concourse.mybir as mybir
from concourse.tile import TileContext

DIM = 1024
H = 16
HD = 64
ROT = 32
B = 4
QL = 2048
KL = 2048
G = 2                # head-group (tensor-parallel) factor
HL = H // G          # 8 local heads
DL = HL * HD         # 512 local feature dims
NPAIR = HL // 2      # 4 head pairs -> 4 [128, T] activation tiles
NCORE = 8
SCI = 8.0            # fp8 input pre-scale (q/k)
SCW = 16.0           # fp8 weight pre-scale (Wq/Wk)
DESC = 1.0 / (SCI * SCW)
LAG = 8              # attn stream lag behind the score stream, in ticks

F32 = mybir.dt.float32
F8 = mybir.dt.float8e4
BF16 = mybir.dt.bfloat16
AFT = mybir.ActivationFunctionType
ALU = mybir.AluOpType
DRMODE = mybir.MatmulPerfMode.DoubleRow
bf16 = ml_dtypes.bfloat16
f8e4 = ml_dtypes.float8_e4m3

_NC_CACHE = {}


def _rot_patterns():
    """cc/ss blend patterns [128, QL] for the head-pair layout, carrying the
    1/128 fp8 descale (pass dims get cc=1/128, ss=0)."""
    inv_freq = 1.0 / (10000.0 ** (np.arange(0, ROT, 2, dtype=np.float64) / ROT))
    t = np.arange(QL, dtype=np.float64)
    freqs = t[:, None] * inv_freq[None, :]          # [QL, 16]
    cos_p = np.ones((HD, QL), np.float64)
    sin_p = np.zeros((HD, QL), np.float64)
    for d in range(ROT):
        j = d // 2
        cos_p[d] = np.cos(freqs[:, j])
        sin_p[d] = np.sin(freqs[:, j]) * (-1.0 if d % 2 == 0 else 1.0)
    cc = np.tile(cos_p, (2, 1)) * DESC              # [128, QL]
    ss = np.tile(sin_p, (2, 1)) * DESC
    return cc.astype(np.float32), ss.astype(np.float32)


def _build_nc():
    if "nc" in _NC_CACHE:
        return _NC_CACHE["nc"]
    nc = bacc.Bacc("TRN2", target_bir_lowering=False)

    d = {}
    for name, shape, dt in [
        # fp8 staging: [128, (a2 4, i 2, T)] with x[p, a2, i, t] = row (a2*2+i)*128+p
        ("q8", [128, 8 * QL], F8), ("k8", [128, 8 * KL], F8),
        ("wq8", [128, 8 * DL], F8), ("wk8", [128, 8 * DL], F8),
        ("vT", [DIM, KL], BF16), ("wvT", [DIM, DL], BF16),
        ("woT", [DL, DIM], BF16),
        ("bqp", [128, NPAIR], F32), ("bkp", [128, NPAIR], F32),
        ("bv", [1, DL], BF16), ("ones1", [1, 128], BF16),
        ("cc", [128, QL], BF16), ("ss", [128, QL], BF16),
    ]:
        d[name] = nc.dram_tensor(name, shape, dt, kind="ExternalInput")
    out_d = nc.dram_tensor("out", [QL, DIM], F32, kind="ExternalOutput")

    vT_t = d["vT"].rearrange("(a p) n -> a p n", p=128)
    wvT_t = d["wvT"].rearrange("(a p) n -> a p n", p=128)
    woT_t = d["woT"].rearrange("(a p) n -> a p n", p=128)   # [4, 128, DIM]
    out_t = out_d.rearrange("(a p) n -> a p n", p=128)      # [16, 128, DIM]
    q8_t = d["q8"].rearrange("p (a x) -> a p x", a=4)       # [4, 128, 2*QL]
    k8_t = d["k8"].rearrange("p (a x) -> a p x", a=4)
    wq8_t = d["wq8"].rearrange("p (a x) -> a p x", a=4)     # [4, 128, 2*DL]
    wk8_t = d["wk8"].rearrange("p (a x) -> a p x", a=4)

    SWAP_MASK = [(j + 1 if j % 2 == 0 else j - 1) for j in range(32)]

    with TileContext(nc) as tc, ExitStack() as top:
        consts = top.enter_context(tc.tile_pool(name="consts", bufs=1))
        bq_s = consts.tile([128, NPAIR], F32)
        nc.scalar.dma_start(out=bq_s, in_=d["bqp"][:, :])
        bk_s = consts.tile([128, NPAIR], F32)
        nc.scalar.dma_start(out=bk_s, in_=d["bkp"][:, :])
        bv_s = consts.tile([1, DL], BF16)
        nc.scalar.dma_start(out=bv_s, in_=d["bv"][:, :])
        ones_s = consts.tile([1, 128], BF16)
        nc.scalar.dma_start(out=ones_s, in_=d["ones1"][:, :])
        cc_s = consts.tile([128, QL], BF16)
        nc.scalar.dma_start(out=cc_s, in_=d["cc"][:, :])
        ss_s = consts.tile([128, QL], BF16)
        nc.scalar.dma_start(out=ss_s, in_=d["ss"][:, :])
        wo_s = [consts.tile([128, DIM], BF16, tag=f"wo{i}", name=f"wo{i}")
                for i in range(NPAIR)]
        for i in range(NPAIR):
            nc.scalar.dma_start(out=wo_s[i], in_=woT_t[i])
        warm = consts.tile([1, 8], F32)
        nc.scalar.activation(out=warm, in_=ones_s[0:1, 0:8], func=AFT.Exp)

        # ---- persistent activations ----
        qh_pool = top.enter_context(tc.tile_pool(name="qh", bufs=NPAIR))
        kh_pool = top.enter_context(tc.tile_pool(name="kh", bufs=NPAIR))
        qhT = [qh_pool.tile([128, QL], BF16, tag="qh", name=f"qh{i}")
               for i in range(NPAIR)]
        khT = [kh_pool.tile([128, KL], BF16, tag="kh", name=f"kh{i}")
               for i in range(NPAIR)]
        vh_pool = top.enter_context(tc.tile_pool(name="vh", bufs=16))
        vh = [vh_pool.tile([128, NPAIR * 130], BF16, tag="vh", name=f"vh{i}")
              for i in range(16)]
        at_pool = top.enter_context(tc.tile_pool(name="atn", bufs=NPAIR))
        apT = [at_pool.tile([128, QL], BF16, tag="at", name=f"apT{i}")
               for i in range(NPAIR)]

        with ExitStack() as ph:
            # shared pool: fp8 q/k staging first, ets pair-tiles afterwards
            big = ph.enter_context(tc.tile_pool(name="big", bufs=12))
            vstage = ph.enter_context(tc.tile_pool(name="vstage", bufs=8))
            w8_p = ph.enter_context(tc.tile_pool(name="w8p", bufs=8))
            wv_p = ph.enter_context(tc.tile_pool(name="wvp", bufs=8))
            sw_p = ph.enter_context(tc.tile_pool(name="swp", bufs=1))
            t2_p = ph.enter_context(tc.tile_pool(name="t2p", bufs=1))
            atu_p = ph.enter_context(tc.tile_pool(name="atu", bufs=2))
            rc_p = ph.enter_context(tc.tile_pool(name="rcp", bufs=2))
            bt_p = ph.enter_context(tc.tile_pool(name="btp", bufs=2))
            out_p = ph.enter_context(tc.tile_pool(name="outp", bufs=1))
            dscr = ph.enter_context(tc.tile_pool(name="dscr", bufs=8, space="DRAM"))
            psS = ph.enter_context(tc.tile_pool(name="psS", bufs=3, space="PSUM"))
            psPA = ph.enter_context(tc.tile_pool(name="psPA", bufs=1, space="PSUM"))

            # ---- input staging DMAs ----
            ks8 = [big.tile([128, 2, KL], F8, tag="big", name=f"k8_{a}")
                   for a in range(4)]
            for a in range(4):
                nc.sync.dma_start(
                    out=ks8[a],
                    in_=k8_t[a].rearrange("p (i n) -> p i n", i=2))
            qs8 = [big.tile([128, 2, QL], F8, tag="big", name=f"q8_{a}")
                   for a in range(4)]
            for a in range(4):
                nc.sync.dma_start(
                    out=qs8[a],
                    in_=q8_t[a].rearrange("p (i n) -> p i n", i=2))
            wk8 = [w8_p.tile([128, 2, DL], F8, tag="w8", name=f"wk8_{a}")
                   for a in range(4)]
            for a in range(4):
                nc.scalar.dma_start(
                    out=wk8[a],
                    in_=wk8_t[a].rearrange("p (i n) -> p i n", i=2))
            wq8 = [w8_p.tile([128, 2, DL], F8, tag="w8", name=f"wq8_{a}")
                   for a in range(4)]
            for a in range(4):
                nc.scalar.dma_start(
                    out=wq8[a],
                    in_=wq8_t[a].rearrange("p (i n) -> p i n", i=2))
            vs = [vstage.tile([128, KL], BF16, tag="vst", name=f"vs{a}")
                  for a in range(8)]
            for a in range(8):
                nc.gpsimd.dma_start(out=vs[a], in_=vT_t[a])
            wvs = [wv_p.tile([128, DL], BF16, tag="wv", name=f"wvs{a}")
                   for a in range(8)]
            for a in range(8):
                nc.scalar.dma_start(out=wvs[a], in_=wvT_t[a])

            # ---- fp8 DoubleRow K/Q projection + rotary (lead-in) ----
            def rotary(dst, mt):
                for c2 in range(2):
                    cs = slice(c2 * 1024, (c2 + 1) * 1024)
                    qt = dst[mt][:, cs]
                    sw = sw_p.tile([128, 1024], BF16, tag="sw")
                    nc.vector.stream_shuffle(out=sw, in_=qt, mask=SWAP_MASK)
                    nc.vector.tensor_tensor(out=sw, in0=sw, in1=ss_s[:, cs],
                                            op=ALU.mult)
                    t2 = t2_p.tile([128, 1024], BF16, tag="t2")
                    nc.vector.tensor_tensor(out=t2, in0=qt, in1=cc_s[:, cs],
                                            op=ALU.mult)
                    nc.vector.tensor_tensor(out=qt, in0=sw, in1=t2,
                                            op=ALU.add)

            def qkproj(xs8, ws8, b_s, dst, mt):
                """One pair-tile projection: DR chains + bias, then rotary."""
                for c2 in range(2):
                    ps = psS.tile([128, 1024], F32, tag="S", name=f"pj{mt}{c2}")
                    for a in range(4):
                        for n in range(2):
                            nc.tensor.matmul(
                                ps[:, n * 512:(n + 1) * 512],
                                lhsT=ws8[a][:, :, mt * 128:(mt + 1) * 128],
                                rhs=xs8[a][:, :, c2 * 1024 + n * 512:
                                           c2 * 1024 + (n + 1) * 512],
                                start=(a == 0), stop=(a == 3),
                                perf_mode=DRMODE,
                            )
                    nc.vector.tensor_scalar_add(
                        out=dst[mt][:, c2 * 1024:(c2 + 1) * 1024], in0=ps,
                        scalar1=b_s[:, mt:mt + 1])
                rotary(dst, mt)

            for mt in range(NPAIR):
                qkproj(ks8, wk8, bk_s, khT, mt)
            for mt in range(NPAIR):
                qkproj(qs8, wq8, bq_s, qhT, mt)

            # ---- bf16 V projection (lead-in) ----
            for t in range(16):
                ps = psS.tile([128, 512], F32, tag="S", name=f"vp{t}")
                for a in range(8):
                    nc.tensor.matmul(
                        ps, lhsT=vs[a][:, t * 128:(t + 1) * 128], rhs=wvs[a],
                        start=(a == 0), stop=False)
                nc.tensor.matmul(ps, lhsT=ones_s, rhs=bv_s,
                                 start=False, stop=True)
                vtr = vh[t].rearrange("p (g h e) -> p g h e", h=2, e=65)
                nc.vector.memset(vtr[:, :, :, 64:65], 1.0)
                psr = ps.rearrange("p (g h e) -> p g h e", h=2, e=64)
                nc.vector.tensor_copy(out=vtr[:, :, :, 0:64], in_=psr)

            # ---- boundary chain queue (out-projection) ----
            fillers = deque()

            def pull(n=1):
                for _ in range(n):
                    if fillers:
                        fillers.popleft()()

            def outproj_closures(qts):
                out = []
                state = {}

                def chain(qt):
                    def go():
                        state[qt] = psPA.tile([128, 1024], F32, tag="PA",
                                              name=f"op{qt}")
                        ps = state[qt]
                        for dc in range(2):
                            for p in range(NPAIR):
                                nc.tensor.matmul(
                                    ps[:, dc * 512:(dc + 1) * 512],
                                    lhsT=apT[p][:, qt * 128:(qt + 1) * 128],
                                    rhs=wo_s[p][:, dc * 512:(dc + 1) * 512],
                                    start=(p == 0), stop=(p == NPAIR - 1),
                                )
                        ot = out_p.tile([128, DIM], F32, tag="o")
                        nc.vector.tensor_copy(out=ot, in_=ps)
                        nc.sync.dma_start(out=out_t[qt], in_=ot)
                    return go

                for qt in qts:
                    out.append(chain(qt))
                return out

            def normalize(pa, u):
                qc, p, hh = u // 8, (u % 8) // 2, u % 2
                atu = atu_p.tile([128, 1024], F32, tag="atu")
                nc.vector.tensor_copy(out=atu[0:65, :], in_=pa[0:65, :])
                ds = dscr.tile([1, 1024], F32, tag="dsc")
                nc.sync.dma_start(out=ds, in_=atu[64:65, :])
                rc8 = rc_p.tile([128, 8], F32, tag="rc8")
                nc.sync.dma_start(out=rc8,
                                  in_=ds.rearrange("a (p e) -> (a p) e", p=128))
                rc8b = rc_p.tile([128, 8], BF16, tag="rc8b")
                with nc.allow_low_precision(
                        reason="softmax denominators; bf16 reciprocal adds "
                               "~0.4% which is within the error budget"):
                    nc.vector.reciprocal(out=rc8b, in_=rc8)
                ds2 = dscr.tile([1, 1024], BF16, tag="ds2")
                nc.sync.dma_start(
                    out=ds2.rearrange("a (p e) -> (a p) e", p=128), in_=rc8b)
                bt = bt_p.tile([64, 1024], BF16, tag="bc")
                nc.sync.dma_start(out=bt,
                                  in_=ds2[0:1, :].to_broadcast([64, 1024]))
                nc.vector.tensor_tensor(
                    out=apT[p][hh * 64:(hh + 1) * 64,
                               qc * 1024:(qc + 1) * 1024],
                    in0=atu[0:64, :], in1=bt[0:64, :], op=ALU.mult)

            # ---- score stream ticks + lagged attn stream ----
            # unit u = (qc, p, h): qc = u//8, p = (u%8)//2, h = u%2
            ets = {}           # tick -> (pair tile, half slice)
            pa_cur = [None]

            def s_tick(t):
                u, mt = divmod(t, 16)
                qc, p, hh = u // 8, (u % 8) // 2, u % 2
                ps = psS.tile([128, 1024], F32, tag="S", name=f"s{t}")
                for n in range(2):
                    nc.tensor.matmul(
                        ps[:, n * 512:(n + 1) * 512],
                        lhsT=khT[p][hh * 64:(hh + 1) * 64,
                                    mt * 128:(mt + 1) * 128],
                        rhs=qhT[p][hh * 64:(hh + 1) * 64,
                                   qc * 1024 + n * 512:
                                   qc * 1024 + (n + 1) * 512],
                        start=True, stop=True,
                        tile_position=(hh * 64, 0),
                    )
                if mt % 2 == 0:
                    pair = big.tile([128, 2048], BF16, tag="big",
                                    name=f"ep{t}")
                    ets[t] = (pair, slice(0, 1024))
                    ets[t + 1] = (pair, slice(1024, 2048))
                e_tile, e_sl = ets[t]
                nc.scalar.activation(out=e_tile[:, e_sl], in_=ps,
                                     func=AFT.Exp, scale=0.125)

            def a_tick(a):
                u, mt = divmod(a, 16)
                qc, p, hh = u // 8, (u % 8) // 2, u % 2
                if mt == 0:
                    pa_cur[0] = psPA.tile([128, 1024], F32, tag="PA",
                                          name=f"pa{u}")
                pa = pa_cur[0]
                e_tile, e_sl = ets.pop(a)
                lhs = vh[mt][:, p * 130 + hh * 65: p * 130 + (hh + 1) * 65]
                base = e_sl.start
                for n in range(2):
                    nc.tensor.matmul(
                        pa[0:65, n * 512:(n + 1) * 512],
                        lhsT=lhs,
                        rhs=e_tile[:, base + n * 512: base + (n + 1) * 512],
                        start=(mt == 0), stop=(mt == 15),
                    )
                if mt == 15:
                    normalize(pa, u)
                    if u == 7:            # qc=0 attn complete
                        fillers.extend(outproj_closures(range(8)))

            for t in range(256):
                s_tick(t)
                if t >= LAG:
                    a = t - LAG
                    a_tick(a)
                    if a % 16 == 15:
                        pull(1)       # boundary: out-projection chain
            for a in range(256 - LAG, 256):
                a_tick(a)
                pull(1)
            while fillers:
                pull(1)
            for fn in outproj_closures(range(8, 16)):
                fn()

    nc.compile()
    _NC_CACHE["nc"] = nc
    return nc


def _pack_fp8_pairs(mat_T, scale):
    """[1024, C] f32 -> [128, 4, 2, C] fp8 with x[p, a2, i, c] = row (a2*2+i)*128+p."""
    C = mat_T.shape[1]
    x = np.clip(mat_T * scale, -240, 240).astype(f8e4)
    x = x.reshape(4, 2, 128, C).transpose(2, 0, 1, 3)     # [128, 4, 2, C]
    return np.ascontiguousarray(x.reshape(128, 8 * C))


def _make_in_maps(q, k, v, Wq, bq, Wk, bk, Wv, bv, Wo, bo):
    q, k, v = (np.asarray(x, np.float32) for x in (q, k, v))
    Wq, Wk, Wv, Wo = (np.asarray(x, np.float32) for x in (Wq, Wk, Wv, Wo))
    bq, bk, bv, bo = (np.asarray(x, np.float32) for x in (bq, bk, bv, bo))
    cc, ss = _rot_patterns()
    ones1 = np.ones((1, 128), np.float32)
    in_maps = []
    for c in range(NCORE):
        b, g = divmod(c, G)
        gs = slice(g * DL, (g + 1) * DL)
        in_maps.append({
            "q8": _pack_fp8_pairs(np.ascontiguousarray(q[b].T), SCI),
            "k8": _pack_fp8_pairs(np.ascontiguousarray(k[b].T), SCI),
            "wq8": _pack_fp8_pairs(np.ascontiguousarray(Wq[gs, :].T), SCW),
            "wk8": _pack_fp8_pairs(np.ascontiguousarray(Wk[gs, :].T), SCW),
            "vT": np.ascontiguousarray(v[b].T).astype(bf16),
            "wvT": np.ascontiguousarray(Wv[gs, :].T).astype(bf16),
            "woT": np.ascontiguousarray(Wo[:, gs].T).astype(bf16),
            # biases pre-scaled by 128 (descale folded into cc/ss)
            "bqp": np.ascontiguousarray(
                (bq[gs] / DESC).reshape(NPAIR, 128).T),
            "bkp": np.ascontiguousarray(
                (bk[gs] / DESC).reshape(NPAIR, 128).T),
            "bv": np.ascontiguousarray(bv[gs][None, :]).astype(bf16),
            "ones1": ones1.astype(bf16),
            "cc": cc.astype(bf16), "ss": ss.astype(bf16),
        })
    return in_maps


def run(inputs: dict, trace: bool = False, tmpdir: str | None = None):
    """Returns (out [B, QL, DIM] f32, exec_time_ns or None)."""
    from concourse.bass_utils import run_bass_kernel_spmd

    nc = _build_nc()
    in_maps = _make_in_maps(**inputs)
    res = run_bass_kernel_spmd(nc, in_maps, list(range(NCORE)), trace=trace,
                               tmpdir=tmpdir)
    globals()["LAST_RES"] = res
    bo = np.asarray(inputs["bo"], np.float32)
    outs = [res.results[i]["out"] for i in range(NCORE)]
    out = np.stack([outs[G * b] + outs[G * b + 1] for b in range(B)])
    out += bo[None, None, :]
    return out.astype(np.float32), res.exec_time_ns


def kernel(**inputs) -> np.ndarray:
    out, _ = run(inputs, trace=False)
    return out


# revision 19
# speedup vs baseline: 1.5792x; 1.0036x over previous
"""Trainium2 Bass kernel for nn_MultiHeadCrossAttention.

Sharding: 8 cores = 4 batches x 2 head-groups (8 local heads each).
Per-core pipeline:
  - Q/K projections via fp8e4 DoubleRow matmuls (K=128 partitions x 2 slots
    per instruction = 2 contraction tiles, full 2.4GHz rate). Host pre-scales
    inputs x8 and weights x16; the 1/128 descale is folded into the rotary
    cos/sin blend patterns, biases are pre-scaled x128.
  - V projection and everything downstream in bf16 (precision budget:
    fp8 q/k projection alone costs ~1.25e-2 rel err, within the 2e-2 gate).
  - Partial rotary via 32-lane stream_shuffle blend (head-pair layout).
  - scores^T per head: K=64 bf16 matmuls quadrant-packed via tile_position,
    alternating heads so adjacent instructions overlap in the PE array.
  - exp on ACT over [128,1024] PSUM tiles (triple-buffered so PE idle clumps
    into long runs that hold the PE at its ramped clock).
  - attn@V in bf16 with an appended ones column giving softmax denominators;
    normalize via reciprocal + DMA partition-broadcast; out-projection is
    row-split; host sums the two head-group partials and adds the bias.
Emission: lead-in (staging DMAs, fp8 K/Q projections + rotary, bf16 V
projection), then a 256-tick score stream (2 score matmuls + 1 exp per tick)
with the attention stream trailing by LAG ticks; out-projection chains are
dosed at unit boundaries so ACT stays saturated end-to-end.
"""

import sys

sys.path.insert(0, "/opt/trn_rl_repo")

from collections import deque

import numpy as np
import ml_dtypes
from contextlib import ExitStack

import concourse.bass as bass
import concourse.bacc as bacc
import concourse.mybir as mybir
from concourse.tile import TileContext

DIM = 1024
H = 16
HD = 64
ROT = 32
B = 4
QL = 2048
KL = 2048
G = 2                # head-group (tensor-parallel) factor
HL = H // G          # 8 local heads
DL = HL * HD         # 512 local feature dims
NPAIR = HL // 2      # 4 head pairs -> 4 [128, T] activation tiles
NCORE = 8
SCI = 8.0            # fp8 input pre-scale (q/k)
SCW = 16.0           # fp8 weight pre-scale (Wq/Wk)
DESC = 1.0 / (SCI * SCW)
LAG = 8              # attn stream lag behind the score stream, in ticks

F32 = mybir.dt.float32
F8 = mybir.dt.float8e4
BF16 = mybir.dt.bfloat16
AFT = mybir.ActivationFunctionType
ALU = mybir.AluOpType
DRMODE = mybir.MatmulPerfMode.DoubleRow
bf16 = ml_dtypes.bfloat16
f8e4 = ml_dtypes.float8_e4m3

_NC_CACHE = {}


def _rot_patterns():
    """cc/ss blend patterns [128, QL] for the head-pair layout, carrying the
    1/128 fp8 descale (pass dims get cc=1/128, ss=0)."""
    inv_freq = 1.0 / (10000.0 ** (np.arange(0, ROT, 2, dtype=np.float64) / ROT))
    t = np.arange(QL, dtype=np.float64)
    freqs = t[:, None] * inv_freq[None, :]          # [QL, 16]
    cos_p = np.ones((HD, QL), np.float64)
    sin_p = np.zeros((HD, QL), np.float64)
    for d in range(ROT):
        j = d // 2
        cos_p[d] = np.cos(freqs[:, j])
        sin_p[d] = np.sin(freqs[:, j]) * (-1.0 if d % 2 == 0 else 1.0)
    cc = np.tile(cos_p, (2, 1)) * DESC              # [128, QL]
    ss = np.tile(sin_p, (2, 1)) * DESC
    return cc.astype(np.float32), ss.astype(np.float32)


def _build_nc():
    if "nc" in _NC_CACHE:
        return _NC_CACHE["nc"]
    nc = bacc.Bacc("TRN2", target_bir_lowering=False)

    d = {}
    for name, shape, dt in [
        # fp8 staging: [128, (a2 4, i 2, T)] with x[p, a2, i, t] = row (a2*2+i)*128+p
        ("q8", [128, 8 * QL], F8), ("k8", [128, 8 * KL], F8),
        ("wq8", [128, 8 * DL], F8), ("wk8", [128, 8 * DL], F8),
        ("vT", [DIM, KL], BF16), ("wvT", [DIM, DL], BF16),
        ("woT", [DL, DIM], BF16),
        ("bqp", [128, NPAIR], F32), ("bkp", [128, NPAIR], F32),
        ("bv", [1, DL], BF16), ("ones1", [1, 128], BF16),
        ("cc", [128, QL], BF16), ("ss", [128, QL], BF16),
    ]:
        d[name] = nc.dram_tensor(name, shape, dt, kind="ExternalInput")
    out_d = nc.dram_tensor("out", [QL, DIM], F32, kind="ExternalOutput")

    vT_t = d["vT"].rearrange("(a p) n -> a p n", p=128)
    wvT_t = d["wvT"].rearrange("(a p) n -> a p n", p=128)
    woT_t = d["woT"].rearrange("(a p) n -> a p n", p=128)   # [4, 128, DIM]
    out_t = out_d.rearrange("(a p) n -> a p n", p=128)      # [16, 128, DIM]
    q8_t = d["q8"].rearrange("p (a x) -> a p x", a=4)       # [4, 128, 2*QL]
    k8_t = d["k8"].rearrange("p (a x) -> a p x", a=4)
    wq8_t = d["wq8"].rearrange("p (a x) -> a p x", a=4)     # [4, 128, 2*DL]
    wk8_t = d["wk8"].rearrange("p (a x) -> a p x", a=4)

    SWAP_MASK = [(j + 1 if j % 2 == 0 else j - 1) for j in range(32)]

    with TileContext(nc) as tc, ExitStack() as top:
        # DMA priority: the critical-path tensors (k8/wk8, then q8/wq8 and the
        # rotary patterns) go first on their queues; wv/wo/v are needed only
        # once the tick stream is running.
        consts = top.enter_context(tc.tile_pool(name="consts", bufs=1))
        bq_s = consts.tile([128, NPAIR], F32)
        bk_s = consts.tile([128, NPAIR], F32)
        bv_s = consts.tile([1, DL], BF16)
        ones_s = consts.tile([1, 128], BF16)
        cc_s = consts.tile([128, QL], BF16)
        ss_s = consts.tile([128, QL], BF16)
        wo_s = [consts.tile([128, DIM], BF16, tag=f"wo{i}", name=f"wo{i}")
                for i in range(NPAIR)]
        warm = consts.tile([1, 8], F32)

        # ---- persistent activations ----
        qh_pool = top.enter_context(tc.tile_pool(name="qh", bufs=NPAIR))
        kh_pool = top.enter_context(tc.tile_pool(name="kh", bufs=NPAIR))
        qhT = [qh_pool.tile([128, QL], BF16, tag="qh", name=f"qh{i}")
               for i in range(NPAIR)]
        khT = [kh_pool.tile([128, KL], BF16, tag="kh", name=f"kh{i}")
               for i in range(NPAIR)]
        vh_pool = top.enter_context(tc.tile_pool(name="vh", bufs=16))
        vh = [vh_pool.tile([128, NPAIR * 130], BF16, tag="vh", name=f"vh{i}")
              for i in range(16)]
        at_pool = top.enter_context(tc.tile_pool(name="atn", bufs=NPAIR))
        apT = [at_pool.tile([128, QL], BF16, tag="at", name=f"apT{i}")
               for i in range(NPAIR)]

        with ExitStack() as ph:
            # shared pool: fp8 q/k staging first, ets pair-tiles afterwards
            big = ph.enter_context(tc.tile_pool(name="big", bufs=12))
            vstage = ph.enter_context(tc.tile_pool(name="vstage", bufs=8))
            w8_p = ph.enter_context(tc.tile_pool(name="w8p", bufs=8))
            wv_p = ph.enter_context(tc.tile_pool(name="wvp", bufs=8))
            sw_p = ph.enter_context(tc.tile_pool(name="swp", bufs=1))
            t2_p = ph.enter_context(tc.tile_pool(name="t2p", bufs=1))
            atu_p = ph.enter_context(tc.tile_pool(name="atu", bufs=2))
            rc_p = ph.enter_context(tc.tile_pool(name="rcp", bufs=2))
            bt_p = ph.enter_context(tc.tile_pool(name="btp", bufs=2))
            out_p = ph.enter_context(tc.tile_pool(name="outp", bufs=1))
            dscr = ph.enter_context(tc.tile_pool(name="dscr", bufs=8, space="DRAM"))
            psS = ph.enter_context(tc.tile_pool(name="psS", bufs=3, space="PSUM"))
            psPA = ph.enter_context(tc.tile_pool(name="psPA", bufs=1, space="PSUM"))

            # ---- input staging DMAs (critical path first) ----
            ks8 = [big.tile([128, 2, KL], F8, tag="big", name=f"k8_{a}")
                   for a in range(4)]
            for a in range(4):
                nc.sync.dma_start(
                    out=ks8[a],
                    in_=k8_t[a].rearrange("p (i n) -> p i n", i=2))
            wk8 = [w8_p.tile([128, 2, DL], F8, tag="w8", name=f"wk8_{a}")
                   for a in range(4)]
            for a in range(4):
                nc.scalar.dma_start(
                    out=wk8[a],
                    in_=wk8_t[a].rearrange("p (i n) -> p i n", i=2))
            nc.scalar.dma_start(out=bk_s, in_=d["bkp"][:, :])
            nc.scalar.dma_start(out=bq_s, in_=d["bqp"][:, :])
            nc.scalar.dma_start(out=ones_s, in_=d["ones1"][:, :])
            nc.scalar.dma_start(out=cc_s, in_=d["cc"][:, :])
            nc.scalar.dma_start(out=ss_s, in_=d["ss"][:, :])
            nc.scalar.activation(out=warm, in_=ones_s[0:1, 0:8], func=AFT.Exp)
            qs8 = [big.tile([128, 2, QL], F8, tag="big", name=f"q8_{a}")
                   for a in range(4)]
            for a in range(4):
                nc.sync.dma_start(
                    out=qs8[a],
                    in_=q8_t[a].rearrange("p (i n) -> p i n", i=2))
            wq8 = [w8_p.tile([128, 2, DL], F8, tag="w8", name=f"wq8_{a}")
                   for a in range(4)]
            for a in range(4):
                nc.scalar.dma_start(
                    out=wq8[a],
                    in_=wq8_t[a].rearrange("p (i n) -> p i n", i=2))
            vs = [vstage.tile([128, KL], BF16, tag="vst", name=f"vs{a}")
                  for a in range(8)]
            for a in range(8):
                nc.gpsimd.dma_start(out=vs[a], in_=vT_t[a])
            wvs = [wv_p.tile([128, DL], BF16, tag="wv", name=f"wvs{a}")
                   for a in range(8)]
            for a in range(8):
                nc.scalar.dma_start(out=wvs[a], in_=wvT_t[a])
            nc.scalar.dma_start(out=bv_s, in_=d["bv"][:, :])
            for i in range(NPAIR):
                nc.scalar.dma_start(out=wo_s[i], in_=woT_t[i])

            # ---- fp8 DoubleRow K/Q projection + rotary (lead-in) ----
            def rotary(dst, mt):
                for c2 in range(2):
                    cs = slice(c2 * 1024, (c2 + 1) * 1024)
                    qt = dst[mt][:, cs]
                    sw = sw_p.tile([128, 1024], BF16, tag="sw")
                    nc.vector.stream_shuffle(out=sw, in_=qt, mask=SWAP_MASK)
                    nc.vector.tensor_tensor(out=sw, in0=sw, in1=ss_s[:, cs],
                                            op=ALU.mult)
                    t2 = t2_p.tile([128, 1024], BF16, tag="t2")
                    nc.vector.tensor_tensor(out=t2, in0=qt, in1=cc_s[:, cs],
                                            op=ALU.mult)
                    nc.vector.tensor_tensor(out=qt, in0=sw, in1=t2,
                                            op=ALU.add)

            def qkproj(xs8, ws8, b_s, dst, mt):
                """One pair-tile projection: DR chains + bias, then rotary."""
                for c2 in range(2):
                    ps = psS.tile([128, 1024], F32, tag="S", name=f"pj{mt}{c2}")
                    for a in range(4):
                        for n in range(2):
                            nc.tensor.matmul(
                                ps[:, n * 512:(n + 1) * 512],
                                lhsT=ws8[a][:, :, mt * 128:(mt + 1) * 128],
                                rhs=xs8[a][:, :, c2 * 1024 + n * 512:
                                           c2 * 1024 + (n + 1) * 512],
                                start=(a == 0), stop=(a == 3),
                                perf_mode=DRMODE,
                            )
                    nc.vector.tensor_scalar_add(
                        out=dst[mt][:, c2 * 1024:(c2 + 1) * 1024], in0=ps,
                        scalar1=b_s[:, mt:mt + 1])
                rotary(dst, mt)

            for mt in range(NPAIR):
                qkproj(ks8, wk8, bk_s, khT, mt)
            for mt in range(NPAIR):
                qkproj(qs8, wq8, bq_s, qhT, mt)

            # ---- filler queue: V projection first, out-projection later ----
            fillers = deque()

            def pull(n=1):
                for _ in range(n):
                    if fillers:
                        fillers.popleft()()

            def vproj_closure(t):
                def go():
                    ps = psS.tile([128, 512], F32, tag="S", name=f"vp{t}")
                    for a in range(8):
                        nc.tensor.matmul(
                            ps, lhsT=vs[a][:, t * 128:(t + 1) * 128],
                            rhs=wvs[a], start=(a == 0), stop=False)
                    nc.tensor.matmul(ps, lhsT=ones_s, rhs=bv_s,
                                     start=False, stop=True)
                    vtr = vh[t].rearrange("p (g h e) -> p g h e", h=2, e=65)
                    nc.vector.memset(vtr[:, :, :, 64:65], 1.0)
                    psr = ps.rearrange("p (g h e) -> p g h e", h=2, e=64)
                    nc.vector.tensor_copy(out=vtr[:, :, :, 0:64], in_=psr)
                return go

            fillers.extend(vproj_closure(t) for t in range(16))

            def outproj_closures(qts, pool_tags):
                out = []
                state = {}

                def chain(qt, pool, tag):
                    def go():
                        state[qt] = pool.tile([128, 1024], F32, tag=tag,
                                              name=f"op{qt}")
                        ps = state[qt]
                        for dc in range(2):
                            for p in range(NPAIR):
                                nc.tensor.matmul(
                                    ps[:, dc * 512:(dc + 1) * 512],
                                    lhsT=apT[p][:, qt * 128:(qt + 1) * 128],
                                    rhs=wo_s[p][:, dc * 512:(dc + 1) * 512],
                                    start=(p == 0), stop=(p == NPAIR - 1),
                                )
                        ot = out_p.tile([128, DIM], F32, tag="o")
                        nc.vector.tensor_copy(out=ot, in_=ps)
                        nc.sync.dma_start(out=out_t[qt], in_=ot)
                    return go

                for i, qt in enumerate(qts):
                    pool, tag = pool_tags[i % len(pool_tags)]
                    out.append(chain(qt, pool, tag))
                return out

            def normalize(pa, u):
                qc, p, hh = u // 8, (u % 8) // 2, u % 2
                atu = atu_p.tile([128, 1024], F32, tag="atu")
                nc.vector.tensor_copy(out=atu[0:65, :], in_=pa[0:65, :])
                ds = dscr.tile([1, 1024], F32, tag="dsc")
                nc.sync.dma_start(out=ds, in_=atu[64:65, :])
                rc8 = rc_p.tile([128, 8], F32, tag="rc8")
                nc.sync.dma_start(out=rc8,
                                  in_=ds.rearrange("a (p e) -> (a p) e", p=128))
                rc8b = rc_p.tile([128, 8], BF16, tag="rc8b")
                with nc.allow_low_precision(
                        reason="softmax denominators; bf16 reciprocal adds "
                               "~0.4% which is within the error budget"):
                    nc.vector.reciprocal(out=rc8b, in_=rc8)
                ds2 = dscr.tile([1, 1024], BF16, tag="ds2")
                nc.sync.dma_start(
                    out=ds2.rearrange("a (p e) -> (a p) e", p=128), in_=rc8b)
                bt = bt_p.tile([64, 1024], BF16, tag="bc")
                nc.sync.dma_start(out=bt,
                                  in_=ds2[0:1, :].to_broadcast([64, 1024]))
                nc.vector.tensor_tensor(
                    out=apT[p][hh * 64:(hh + 1) * 64,
                               qc * 1024:(qc + 1) * 1024],
                    in0=atu[0:64, :], in1=bt[0:64, :], op=ALU.mult)

            # ---- score stream ticks + lagged attn stream ----
            # unit u = (qc, p, h): qc = u//8, p = (u%8)//2, h = u%2
            ets = {}           # tick -> (pair tile, half slice)
            pa_cur = [None]

            def s_tick(t):
                u, mt = divmod(t, 16)
                qc, p, hh = u // 8, (u % 8) // 2, u % 2
                ps = psS.tile([128, 1024], F32, tag="S", name=f"s{t}")
                for n in range(2):
                    nc.tensor.matmul(
                        ps[:, n * 512:(n + 1) * 512],
                        lhsT=khT[p][hh * 64:(hh + 1) * 64,
                                    mt * 128:(mt + 1) * 128],
                        rhs=qhT[p][hh * 64:(hh + 1) * 64,
                                   qc * 1024 + n * 512:
                                   qc * 1024 + (n + 1) * 512],
                        start=True, stop=True,
                        tile_position=(hh * 64, 0),
                    )
                if mt % 2 == 0:
                    pair = big.tile([128, 2048], BF16, tag="big",
                                    name=f"ep{t}")
                    ets[t] = (pair, slice(0, 1024))
                    ets[t + 1] = (pair, slice(1024, 2048))
                e_tile, e_sl = ets[t]
                nc.scalar.activation(out=e_tile[:, e_sl], in_=ps,
                                     func=AFT.Exp, scale=0.125)

            def a_tick(a):
                u, mt = divmod(a, 16)
                qc, p, hh = u // 8, (u % 8) // 2, u % 2
                if mt == 0:
                    pa_cur[0] = psPA.tile([128, 1024], F32, tag="PA",
                                          name=f"pa{u}")
                pa = pa_cur[0]
                e_tile, e_sl = ets.pop(a)
                lhs = vh[mt][:, p * 130 + hh * 65: p * 130 + (hh + 1) * 65]
                base = e_sl.start
                for n in range(2):
                    nc.tensor.matmul(
                        pa[0:65, n * 512:(n + 1) * 512],
                        lhsT=lhs,
                        rhs=e_tile[:, base + n * 512: base + (n + 1) * 512],
                        start=(mt == 0), stop=(mt == 15),
                    )
                if mt == 15:
                    normalize(pa, u)
                    if u == 7:            # qc=0 attn complete
                        fillers.extend(
                            outproj_closures(range(8), [(psPA, "PA")]))

            for t in range(256):
                s_tick(t)
                if t < 16:
                    pull(1)           # V projection rides the early ticks
                if t >= LAG:
                    a = t - LAG
                    a_tick(a)
                    if a % 16 == 15:
                        pull(1)       # boundary: out-projection chain
            for a in range(256 - LAG, 256):
                a_tick(a)
                pull(1)
            while fillers:
                pull(1)
            # tail: alternate PSUM pools so chains overlap their copy-out
            for fn in outproj_closures(range(8, 16),
                                       [(psPA, "PA"), (psS, "S")]):
                fn()

    nc.compile()
    _NC_CACHE["nc"] = nc
    return nc


def _pack_fp8_pairs(mat_T, scale):
    """[1024, C] f32 -> [128, 4, 2, C] fp8 with x[p, a2, i, c] = row (a2*2+i)*128+p."""
    C = mat_T.shape[1]
    x = np.clip(mat_T * scale, -240, 240).astype(f8e4)
    x = x.reshape(4, 2, 128, C).transpose(2, 0, 1, 3)     # [128, 4, 2, C]
    return np.ascontiguousarray(x.reshape(128, 8 * C))


def _make_in_maps(q, k, v, Wq, bq, Wk, bk, Wv, bv, Wo, bo):
    q, k, v = (np.asarray(x, np.float32) for x in (q, k, v))
    Wq, Wk, Wv, Wo = (np.asarray(x, np.float32) for x in (Wq, Wk, Wv, Wo))
    bq, bk, bv, bo = (np.asarray(x, np.float32) for x in (bq, bk, bv, bo))
    cc, ss = _rot_patterns()
    ones1 = np.ones((1, 128), np.float32)
    in_maps = []
    for c in range(NCORE):
        b, g = divmod(c, G)
        gs = slice(g * DL, (g + 1) * DL)
        in_maps.append({
            "q8": _pack_fp8_pairs(np.ascontiguousarray(q[b].T), SCI),
            "k8": _pack_fp8_pairs(np.ascontiguousarray(k[b].T), SCI),
            "wq8": _pack_fp8_pairs(np.ascontiguousarray(Wq[gs, :].T), SCW),
            "wk8": _pack_fp8_pairs(np.ascontiguousarray(Wk[gs, :].T), SCW),
            "vT": np.ascontiguousarray(v[b].T).astype(bf16),
            "wvT": np.ascontiguousarray(Wv[gs, :].T).astype(bf16),
            "woT": np.ascontiguousarray(Wo[:, gs].T).astype(bf16),
            # biases pre-scaled by 128 (descale folded into cc/ss)
            "bqp": np.ascontiguousarray(
                (bq[gs] / DESC).reshape(NPAIR, 128).T),
            "bkp": np.ascontiguousarray(
                (bk[gs] / DESC).reshape(NPAIR, 128).T),
            "bv": np.ascontiguousarray(bv[gs][None, :]).astype(bf16),
            "ones1": ones1.astype(bf16),
            "cc": cc.astype(bf16), "ss": ss.astype(bf16),
        })
    return in_maps


def run(inputs: dict, trace: bool = False, tmpdir: str | None = None):
    """Returns (out [B, QL, DIM] f32, exec_time_ns or None)."""
    from concourse.bass_utils import run_bass_kernel_spmd

    nc = _build_nc()
    in_maps = _make_in_maps(**inputs)
    res = run_bass_kernel_spmd(nc, in_maps, list(range(NCORE)), trace=trace,
                               tmpdir=tmpdir)
    globals()["LAST_RES"] = res
    bo = np.asarray(inputs["bo"], np.float32)
    outs = [res.results[i]["out"] for i in range(NCORE)]
    out = np.stack([outs[G * b] + outs[G * b + 1] for b in range(B)])
    out += bo[None, None, :]
    return out.astype(np.float32), res.exec_time_ns


def kernel(**inputs) -> np.ndarray:
    out, _ = run(inputs, trace=False)
    return out


# revision 22
# speedup vs baseline: 1.6821x; 1.0651x over previous
"""Trainium2 Bass kernel for nn_MultiHeadCrossAttention.

Sharding: 8 cores = 4 batches x 2 head-groups (8 local heads each).
Per-core pipeline:
  - Q/K projections via fp8e4 DoubleRow matmuls (K=128 partitions x 2 slots
    per instruction = 2 contraction tiles, full 2.4GHz rate). Host pre-scales
    inputs x8 and weights x16; the 1/128 descale is folded into the rotary
    cos/sin blend patterns, biases are pre-scaled x128.
  - V projection and everything downstream in bf16 (precision budget:
    fp8 q/k projection alone costs ~1.25e-2 rel err, within the 2e-2 gate).
  - Partial rotary via 32-lane stream_shuffle blend (head-pair layout).
  - scores^T per head: K=64 bf16 matmuls quadrant-packed via tile_position,
    alternating heads so adjacent instructions overlap in the PE array.
  - exp on ACT over [128,1024] PSUM tiles (triple-buffered so PE idle clumps
    into long runs that hold the PE at its ramped clock).
  - attn@V in bf16 with an appended ones column giving softmax denominators;
    normalize via reciprocal + DMA partition-broadcast; out-projection is
    row-split; host sums the two head-group partials and adds the bias.
Emission: lead-in (staging DMAs, fp8 K/Q projections + rotary, bf16 V
projection), then a 256-tick score stream (2 score matmuls + 1 exp per tick)
with the attention stream trailing by LAG ticks; out-projection chains are
dosed at unit boundaries so ACT stays saturated end-to-end.
"""

import sys

sys.path.insert(0, "/opt/trn_rl_repo")

from collections import deque

import numpy as np
import ml_dtypes
from contextlib import ExitStack

import concourse.bass as bass
import concourse.bacc as bacc
import concourse.mybir as mybir
from concourse.tile import TileContext

DIM = 1024
H = 16
HD = 64
ROT = 32
B = 4
QL = 2048
KL = 2048
G = 2                # head-group (tensor-parallel) factor
HL = H // G          # 8 local heads
DL = HL * HD         # 512 local feature dims
NPAIR = HL // 2      # 4 head pairs -> 4 [128, T] activation tiles
NCORE = 8
SCI = 8.0            # fp8 input pre-scale (q/k)
SCW = 16.0           # fp8 weight pre-scale (Wq/Wk)
DESC = 1.0 / (SCI * SCW)
LAG = 8              # attn stream lag behind the score stream, in ticks

F32 = mybir.dt.float32
F8 = mybir.dt.float8e4
BF16 = mybir.dt.bfloat16
AFT = mybir.ActivationFunctionType
ALU = mybir.AluOpType
DRMODE = mybir.MatmulPerfMode.DoubleRow
bf16 = ml_dtypes.bfloat16
f8e4 = ml_dtypes.float8_e4m3

_NC_CACHE = {}


def _rot_patterns():
    """cc/ss blend patterns [128, QL] for the head-pair layout, carrying the
    1/128 fp8 descale (pass dims get cc=1/128, ss=0)."""
    inv_freq = 1.0 / (10000.0 ** (np.arange(0, ROT, 2, dtype=np.float64) / ROT))
    t = np.arange(QL, dtype=np.float64)
    freqs = t[:, None] * inv_freq[None, :]          # [QL, 16]
    cos_p = np.ones((HD, QL), np.float64)
    sin_p = np.zeros((HD, QL), np.float64)
    for d in range(ROT):
        j = d // 2
        cos_p[d] = np.cos(freqs[:, j])
        sin_p[d] = np.sin(freqs[:, j]) * (-1.0 if d % 2 == 0 else 1.0)
    cc = np.tile(cos_p, (2, 1)) * DESC              # [128, QL]
    ss = np.tile(sin_p, (2, 1)) * DESC
    return cc.astype(np.float32), ss.astype(np.float32)


def _build_nc():
    if "nc" in _NC_CACHE:
        return _NC_CACHE["nc"]
    nc = bacc.Bacc("TRN2", target_bir_lowering=False)

    d = {}
    for name, shape, dt in [
        # fp8 staging: [128, (a2 4, i 2, T)] with x[p, a2, i, t] = row (a2*2+i)*128+p
        ("q8", [128, 8 * QL], F8), ("k8", [128, 8 * KL], F8),
        ("wq8", [128, 8 * DL], F8), ("wk8", [128, 8 * DL], F8),
        ("vT", [DIM, KL], BF16), ("wvT", [DIM, DL], BF16),
        ("woT", [DL, DIM], BF16),
        ("bqp", [128, NPAIR], F32), ("bkp", [128, NPAIR], F32),
        ("bv", [1, DL], BF16), ("ones1", [1, 128], BF16),
        ("cc", [128, QL], BF16), ("ss", [128, QL], BF16),
    ]:
        d[name] = nc.dram_tensor(name, shape, dt, kind="ExternalInput")
    out_d = nc.dram_tensor("out", [QL, DIM], F32, kind="ExternalOutput")

    vT_t = d["vT"].rearrange("(a p) n -> a p n", p=128)
    wvT_t = d["wvT"].rearrange("(a p) n -> a p n", p=128)
    woT_t = d["woT"].rearrange("(a p) n -> a p n", p=128)   # [4, 128, DIM]
    out_t = out_d.rearrange("(a p) n -> a p n", p=128)      # [16, 128, DIM]
    q8_t = d["q8"].rearrange("p (a x) -> a p x", a=4)       # [4, 128, 2*QL]
    k8_t = d["k8"].rearrange("p (a x) -> a p x", a=4)
    wq8_t = d["wq8"].rearrange("p (a x) -> a p x", a=4)     # [4, 128, 2*DL]
    wk8_t = d["wk8"].rearrange("p (a x) -> a p x", a=4)

    SWAP_MASK = [(j + 1 if j % 2 == 0 else j - 1) for j in range(32)]

    with TileContext(nc) as tc, ExitStack() as top:
        # DMA priority: the critical-path tensors (k8/wk8, then q8/wq8 and the
        # rotary patterns) go first on their queues; wv/wo/v are needed only
        # once the tick stream is running.
        consts = top.enter_context(tc.tile_pool(name="consts", bufs=1))
        bq_s = consts.tile([128, NPAIR], F32)
        bk_s = consts.tile([128, NPAIR], F32)
        bv_s = consts.tile([1, DL], BF16)
        ones_s = consts.tile([1, 128], BF16)
        cc_s = consts.tile([128, QL], BF16)
        ss_s = consts.tile([128, QL], BF16)
        wo_s = [consts.tile([128, DIM], BF16, tag=f"wo{i}", name=f"wo{i}")
                for i in range(NPAIR)]
        warm = consts.tile([1, 8], F32)

        # ---- persistent activations ----
        qh_pool = top.enter_context(tc.tile_pool(name="qh", bufs=NPAIR))
        kh_pool = top.enter_context(tc.tile_pool(name="kh", bufs=NPAIR))
        qhT = [qh_pool.tile([128, QL], BF16, tag="qh", name=f"qh{i}")
               for i in range(NPAIR)]
        khT = [kh_pool.tile([128, KL], BF16, tag="kh", name=f"kh{i}")
               for i in range(NPAIR)]
        vh_pool = top.enter_context(tc.tile_pool(name="vh", bufs=16))
        vh = [vh_pool.tile([128, NPAIR * 130], BF16, tag="vh", name=f"vh{i}")
              for i in range(16)]
        at_pool = top.enter_context(tc.tile_pool(name="atn", bufs=NPAIR))
        apT = [at_pool.tile([128, QL], BF16, tag="at", name=f"apT{i}")
               for i in range(NPAIR)]

        with ExitStack() as ph:
            # shared pool: fp8 q/k staging first, ets pair-tiles afterwards
            big = ph.enter_context(tc.tile_pool(name="big", bufs=11))
            vstage = ph.enter_context(tc.tile_pool(name="vstage", bufs=8))
            w8_p = ph.enter_context(tc.tile_pool(name="w8p", bufs=8))
            wv_p = ph.enter_context(tc.tile_pool(name="wvp", bufs=8))
            sw_p = ph.enter_context(tc.tile_pool(name="swp", bufs=1))
            t2_p = ph.enter_context(tc.tile_pool(name="t2p", bufs=1))
            atu_p = ph.enter_context(tc.tile_pool(name="atu", bufs=2))
            rc_p = ph.enter_context(tc.tile_pool(name="rcp", bufs=2))
            bt_p = ph.enter_context(tc.tile_pool(name="btp", bufs=2))
            out_p = ph.enter_context(tc.tile_pool(name="outp", bufs=2))
            dscr = ph.enter_context(tc.tile_pool(name="dscr", bufs=8, space="DRAM"))
            psS = ph.enter_context(tc.tile_pool(name="psS", bufs=3, space="PSUM"))
            psPA = ph.enter_context(tc.tile_pool(name="psPA", bufs=1, space="PSUM"))

            # ---- input staging DMAs (critical path first) ----
            ks8 = [big.tile([128, 2, KL], F8, tag="big", name=f"k8_{a}")
                   for a in range(4)]
            for a in range(4):
                nc.sync.dma_start(
                    out=ks8[a],
                    in_=k8_t[a].rearrange("p (i n) -> p i n", i=2))
            wk8 = [w8_p.tile([128, 2, DL], F8, tag="w8", name=f"wk8_{a}")
                   for a in range(4)]
            for a in range(4):
                nc.scalar.dma_start(
                    out=wk8[a],
                    in_=wk8_t[a].rearrange("p (i n) -> p i n", i=2))
            nc.scalar.dma_start(out=bk_s, in_=d["bkp"][:, :])
            nc.scalar.dma_start(out=bq_s, in_=d["bqp"][:, :])
            nc.scalar.dma_start(out=ones_s, in_=d["ones1"][:, :])
            nc.scalar.dma_start(out=cc_s, in_=d["cc"][:, :])
            nc.scalar.dma_start(out=ss_s, in_=d["ss"][:, :])
            nc.scalar.activation(out=warm, in_=ones_s[0:1, 0:8], func=AFT.Exp)
            qs8 = [big.tile([128, 2, QL], F8, tag="big", name=f"q8_{a}")
                   for a in range(4)]
            for a in range(4):
                nc.sync.dma_start(
                    out=qs8[a],
                    in_=q8_t[a].rearrange("p (i n) -> p i n", i=2))
            wq8 = [w8_p.tile([128, 2, DL], F8, tag="w8", name=f"wq8_{a}")
                   for a in range(4)]
            for a in range(4):
                nc.scalar.dma_start(
                    out=wq8[a],
                    in_=wq8_t[a].rearrange("p (i n) -> p i n", i=2))
            vs = [vstage.tile([128, KL], BF16, tag="vst", name=f"vs{a}")
                  for a in range(8)]
            for a in range(8):
                nc.gpsimd.dma_start(out=vs[a], in_=vT_t[a])
            wvs = [wv_p.tile([128, DL], BF16, tag="wv", name=f"wvs{a}")
                   for a in range(8)]
            for a in range(8):
                nc.scalar.dma_start(out=wvs[a], in_=wvT_t[a])
            nc.scalar.dma_start(out=bv_s, in_=d["bv"][:, :])
            for i in range(NPAIR):
                nc.scalar.dma_start(out=wo_s[i], in_=woT_t[i])

            # ---- fp8 DoubleRow K/Q projection + rotary (lead-in) ----
            def rotary(dst, mt):
                for c2 in range(2):
                    cs = slice(c2 * 1024, (c2 + 1) * 1024)
                    qt = dst[mt][:, cs]
                    sw = sw_p.tile([128, 1024], BF16, tag="sw")
                    nc.vector.stream_shuffle(out=sw, in_=qt, mask=SWAP_MASK)
                    nc.vector.tensor_tensor(out=sw, in0=sw, in1=ss_s[:, cs],
                                            op=ALU.mult)
                    t2 = t2_p.tile([128, 1024], BF16, tag="t2")
                    nc.vector.tensor_tensor(out=t2, in0=qt, in1=cc_s[:, cs],
                                            op=ALU.mult)
                    nc.vector.tensor_tensor(out=qt, in0=sw, in1=t2,
                                            op=ALU.add)

            def qkproj(xs8, ws8, b_s, dst, mt):
                """One pair-tile projection: DR chains + bias, then rotary."""
                for c2 in range(2):
                    ps = psS.tile([128, 1024], F32, tag="S", name=f"pj{mt}{c2}")
                    for a in range(4):
                        for n in range(2):
                            nc.tensor.matmul(
                                ps[:, n * 512:(n + 1) * 512],
                                lhsT=ws8[a][:, :, mt * 128:(mt + 1) * 128],
                                rhs=xs8[a][:, :, c2 * 1024 + n * 512:
                                           c2 * 1024 + (n + 1) * 512],
                                start=(a == 0), stop=(a == 3),
                                perf_mode=DRMODE,
                            )
                    nc.vector.tensor_scalar_add(
                        out=dst[mt][:, c2 * 1024:(c2 + 1) * 1024], in0=ps,
                        scalar1=b_s[:, mt:mt + 1])
                rotary(dst, mt)

            for mt in range(NPAIR):
                qkproj(ks8, wk8, bk_s, khT, mt)
            for mt in range(NPAIR):
                qkproj(qs8, wq8, bq_s, qhT, mt)

            # ---- filler queue: V projection first, out-projection later ----
            fillers = deque()

            def pull(n=1):
                for _ in range(n):
                    if fillers:
                        fillers.popleft()()

            def vproj_closure(t):
                def go():
                    ps = psS.tile([128, 512], F32, tag="S", name=f"vp{t}")
                    for a in range(8):
                        nc.tensor.matmul(
                            ps, lhsT=vs[a][:, t * 128:(t + 1) * 128],
                            rhs=wvs[a], start=(a == 0), stop=False)
                    nc.tensor.matmul(ps, lhsT=ones_s, rhs=bv_s,
                                     start=False, stop=True)
                    vtr = vh[t].rearrange("p (g h e) -> p g h e", h=2, e=65)
                    nc.gpsimd.memset(vtr[:, :, :, 64:65], 1.0)
                    psr = ps.rearrange("p (g h e) -> p g h e", h=2, e=64)
                    nc.scalar.copy(out=vtr[:, :, :, 0:64], in_=psr)
                return go

            fillers.extend(vproj_closure(t) for t in range(16))

            def outproj_closures(qts, pool_tags):
                out = []
                state = {}

                def chain(qt, pool, tag):
                    def go():
                        state[qt] = pool.tile([128, 1024], F32, tag=tag,
                                              name=f"op{qt}")
                        ps = state[qt]
                        for dc in range(2):
                            for p in range(NPAIR):
                                nc.tensor.matmul(
                                    ps[:, dc * 512:(dc + 1) * 512],
                                    lhsT=apT[p][:, qt * 128:(qt + 1) * 128],
                                    rhs=wo_s[p][:, dc * 512:(dc + 1) * 512],
                                    start=(p == 0), stop=(p == NPAIR - 1),
                                )
                        ot = out_p.tile([128, DIM], F32, tag="o")
                        nc.vector.tensor_copy(out=ot, in_=ps)
                        nc.sync.dma_start(out=out_t[qt], in_=ot)
                    return go

                for i, qt in enumerate(qts):
                    pool, tag = pool_tags[i % len(pool_tags)]
                    out.append(chain(qt, pool, tag))
                return out

            def normalize(pa, u):
                qc, p, hh = u // 8, (u % 8) // 2, u % 2
                atu = atu_p.tile([128, 1024], F32, tag="atu")
                nc.vector.tensor_copy(out=atu[0:65, :], in_=pa[0:65, :])
                ds = dscr.tile([1, 1024], F32, tag="dsc")
                nc.sync.dma_start(out=ds, in_=atu[64:65, :])
                rc8 = rc_p.tile([128, 8], F32, tag="rc8")
                nc.sync.dma_start(out=rc8,
                                  in_=ds.rearrange("a (p e) -> (a p) e", p=128))
                rc8b = rc_p.tile([128, 8], BF16, tag="rc8b")
                with nc.allow_low_precision(
                        reason="softmax denominators; bf16 reciprocal adds "
                               "~0.4% which is within the error budget"):
                    nc.vector.reciprocal(out=rc8b, in_=rc8)
                ds2 = dscr.tile([1, 1024], BF16, tag="ds2")
                nc.sync.dma_start(
                    out=ds2.rearrange("a (p e) -> (a p) e", p=128), in_=rc8b)
                bt = bt_p.tile([64, 1024], BF16, tag="bc")
                nc.sync.dma_start(out=bt,
                                  in_=ds2[0:1, :].to_broadcast([64, 1024]))
                nc.vector.tensor_tensor(
                    out=apT[p][hh * 64:(hh + 1) * 64,
                               qc * 1024:(qc + 1) * 1024],
                    in0=atu[0:64, :], in1=bt[0:64, :], op=ALU.mult)

            # ---- score stream ticks + lagged attn stream ----
            # unit u = (qc, p, h): qc = u//8, p = (u%8)//2, h = u%2
            ets = {}           # tick -> (pair tile, half slice)
            pa_cur = [None]

            def s_tick(t):
                u, mt = divmod(t, 16)
                qc, p, hh = u // 8, (u % 8) // 2, u % 2
                ps = psS.tile([128, 1024], F32, tag="S", name=f"s{t}")
                for n in range(2):
                    nc.tensor.matmul(
                        ps[:, n * 512:(n + 1) * 512],
                        lhsT=khT[p][hh * 64:(hh + 1) * 64,
                                    mt * 128:(mt + 1) * 128],
                        rhs=qhT[p][hh * 64:(hh + 1) * 64,
                                   qc * 1024 + n * 512:
                                   qc * 1024 + (n + 1) * 512],
                        start=True, stop=True,
                        tile_position=(hh * 64, 0),
                    )
                if mt % 2 == 0:
                    pair = big.tile([128, 2048], BF16, tag="big",
                                    name=f"ep{t}")
                    ets[t] = (pair, slice(0, 1024))
                    ets[t + 1] = (pair, slice(1024, 2048))
                e_tile, e_sl = ets[t]
                nc.scalar.activation(out=e_tile[:, e_sl], in_=ps,
                                     func=AFT.Exp, scale=0.125)

            def a_tick(a):
                u, mt = divmod(a, 16)
                qc, p, hh = u // 8, (u % 8) // 2, u % 2
                if mt == 0:
                    pa_cur[0] = psPA.tile([128, 1024], F32, tag="PA",
                                          name=f"pa{u}")
                pa = pa_cur[0]
                e_tile, e_sl = ets.pop(a)
                lhs = vh[mt][:, p * 130 + hh * 65: p * 130 + (hh + 1) * 65]
                base = e_sl.start
                for n in range(2):
                    nc.tensor.matmul(
                        pa[0:65, n * 512:(n + 1) * 512],
                        lhsT=lhs,
                        rhs=e_tile[:, base + n * 512: base + (n + 1) * 512],
                        start=(mt == 0), stop=(mt == 15),
                    )
                if mt == 15:
                    normalize(pa, u)

            for t in range(256):
                s_tick(t)
                if t < 16:
                    pull(1)           # V projection rides the early ticks
                if t >= LAG:
                    a_tick(t - LAG)
            for a in range(256 - LAG, 256):
                a_tick(a)
            # tail: alternate PSUM pools so chains overlap their copy-out
            for fn in outproj_closures(range(16),
                                       [(psPA, "PA"), (psS, "S")]):
                fn()

    nc.compile()
    _NC_CACHE["nc"] = nc
    return nc


def _pack_fp8_pairs(mat_T, scale):
    """[1024, C] f32 -> [128, 4, 2, C] fp8 with x[p, a2, i, c] = row (a2*2+i)*128+p."""
    C = mat_T.shape[1]
    x = np.clip(mat_T * scale, -240, 240).astype(f8e4)
    x = x.reshape(4, 2, 128, C).transpose(2, 0, 1, 3)     # [128, 4, 2, C]
    return np.ascontiguousarray(x.reshape(128, 8 * C))


def _make_in_maps(q, k, v, Wq, bq, Wk, bk, Wv, bv, Wo, bo):
    q, k, v = (np.asarray(x, np.float32) for x in (q, k, v))
    Wq, Wk, Wv, Wo = (np.asarray(x, np.float32) for x in (Wq, Wk, Wv, Wo))
    bq, bk, bv, bo = (np.asarray(x, np.float32) for x in (bq, bk, bv, bo))
    cc, ss = _rot_patterns()
    ones1 = np.ones((1, 128), np.float32)
    in_maps = []
    for c in range(NCORE):
        b, g = divmod(c, G)
        gs = slice(g * DL, (g + 1) * DL)
        in_maps.append({
            "q8": _pack_fp8_pairs(np.ascontiguousarray(q[b].T), SCI),
            "k8": _pack_fp8_pairs(np.ascontiguousarray(k[b].T), SCI),
            "wq8": _pack_fp8_pairs(np.ascontiguousarray(Wq[gs, :].T), SCW),
            "wk8": _pack_fp8_pairs(np.ascontiguousarray(Wk[gs, :].T), SCW),
            "vT": np.ascontiguousarray(v[b].T).astype(bf16),
            "wvT": np.ascontiguousarray(Wv[gs, :].T).astype(bf16),
            "woT": np.ascontiguousarray(Wo[:, gs].T).astype(bf16),
            # biases pre-scaled by 128 (descale folded into cc/ss)
            "bqp": np.ascontiguousarray(
                (bq[gs] / DESC).reshape(NPAIR, 128).T),
            "bkp": np.ascontiguousarray(
                (bk[gs] / DESC).reshape(NPAIR, 128).T),
            "bv": np.ascontiguousarray(bv[gs][None, :]).astype(bf16),
            "ones1": ones1.astype(bf16),
            "cc": cc.astype(bf16), "ss": ss.astype(bf16),
        })
    return in_maps


def run(inputs: dict, trace: bool = False, tmpdir: str | None = None):
    """Returns (out [B, QL, DIM] f32, exec_time_ns or None)."""
    from concourse.bass_utils import run_bass_kernel_spmd

    nc = _build_nc()
    in_maps = _make_in_maps(**inputs)
    res = run_bass_kernel_spmd(nc, in_maps, list(range(NCORE)), trace=trace,
                               tmpdir=tmpdir)
    globals()["LAST_RES"] = res
    bo = np.asarray(inputs["bo"], np.float32)
    outs = [res.results[i]["out"] for i in range(NCORE)]
    out = np.stack([outs[G * b] + outs[G * b + 1] for b in range(B)])
    out += bo[None, None, :]
    return out.astype(np.float32), res.exec_time_ns


def kernel(**inputs) -> np.ndarray:
    out, _ = run(inputs, trace=False)
    return out


# revision 24
# speedup vs baseline: 1.7084x; 1.0156x over previous
"""Trainium2 Bass kernel for nn_MultiHeadCrossAttention.

Sharding: 8 cores = 4 batches x 2 head-groups (8 local heads each).
Per-core pipeline:
  - Q/K projections via fp8e4 DoubleRow matmuls (K=128 partitions x 2 slots
    per instruction = 2 contraction tiles, full 2.4GHz rate). Host pre-scales
    inputs x8 and weights x16; the 1/128 descale is folded into the rotary
    cos/sin blend patterns, biases are pre-scaled x128.
  - V projection and everything downstream in bf16 (precision budget:
    fp8 q/k projection alone costs ~1.25e-2 rel err, within the 2e-2 gate).
  - Partial rotary via 32-lane stream_shuffle blend (head-pair layout).
  - scores^T per head: K=64 bf16 matmuls quadrant-packed via tile_position,
    alternating heads so adjacent instructions overlap in the PE array.
  - exp on ACT over [128,1024] PSUM tiles (triple-buffered so PE idle clumps
    into long runs that hold the PE at its ramped clock).
  - attn@V in bf16 with an appended ones column giving softmax denominators;
    normalize via reciprocal + DMA partition-broadcast; out-projection is
    row-split; host sums the two head-group partials and adds the bias.
Emission: lead-in (staging DMAs, fp8 K/Q projections + rotary, bf16 V
projection), then a 256-tick score stream (2 score matmuls + 1 exp per tick)
with the attention stream trailing by LAG ticks; out-projection chains are
dosed at unit boundaries so ACT stays saturated end-to-end.
"""

import sys

sys.path.insert(0, "/opt/trn_rl_repo")

from collections import deque

import numpy as np
import ml_dtypes
from contextlib import ExitStack

import concourse.bass as bass
import concourse.bacc as bacc
import concourse.mybir as mybir
from concourse.tile import TileContext

DIM = 1024
H = 16
HD = 64
ROT = 32
B = 4
QL = 2048
KL = 2048
G = 2                # head-group (tensor-parallel) factor
HL = H // G          # 8 local heads
DL = HL * HD         # 512 local feature dims
NPAIR = HL // 2      # 4 head pairs -> 4 [128, T] activation tiles
NCORE = 8
SCI = 8.0            # fp8 input pre-scale (q/k)
SCW = 16.0           # fp8 weight pre-scale (Wq/Wk)
DESC = 1.0 / (SCI * SCW)
LAG = 8              # attn stream lag behind the score stream, in ticks

F32 = mybir.dt.float32
F8 = mybir.dt.float8e4
BF16 = mybir.dt.bfloat16
AFT = mybir.ActivationFunctionType
ALU = mybir.AluOpType
DRMODE = mybir.MatmulPerfMode.DoubleRow
bf16 = ml_dtypes.bfloat16
f8e4 = ml_dtypes.float8_e4m3

_NC_CACHE = {}


def _rot_patterns():
    """cc/ss blend patterns [128, QL] for the head-pair layout, carrying the
    1/128 fp8 descale (pass dims get cc=1/128, ss=0)."""
    inv_freq = 1.0 / (10000.0 ** (np.arange(0, ROT, 2, dtype=np.float64) / ROT))
    t = np.arange(QL, dtype=np.float64)
    freqs = t[:, None] * inv_freq[None, :]          # [QL, 16]
    cos_p = np.ones((HD, QL), np.float64)
    sin_p = np.zeros((HD, QL), np.float64)
    for d in range(ROT):
        j = d // 2
        cos_p[d] = np.cos(freqs[:, j])
        sin_p[d] = np.sin(freqs[:, j]) * (-1.0 if d % 2 == 0 else 1.0)
    cc = np.tile(cos_p, (2, 1)) * DESC              # [128, QL]
    ss = np.tile(sin_p, (2, 1)) * DESC
    return cc.astype(np.float32), ss.astype(np.float32)


def _build_nc():
    if "nc" in _NC_CACHE:
        return _NC_CACHE["nc"]
    nc = bacc.Bacc("TRN2", target_bir_lowering=False)

    d = {}
    for name, shape, dt in [
        # fp8 staging: [128, (a2 4, i 2, T)] with x[p, a2, i, t] = row (a2*2+i)*128+p
        ("q8", [128, 8 * QL], F8), ("k8", [128, 8 * KL], F8),
        ("wq8", [128, 8 * DL], F8), ("wk8", [128, 8 * DL], F8),
        ("vT", [DIM, KL], BF16), ("wvT", [DIM, DL], BF16),
        ("woT", [DL, DIM], BF16),
        ("bqp", [128, NPAIR], F32), ("bkp", [128, NPAIR], F32),
        ("bv", [1, DL], BF16), ("ones1", [1, 128], BF16),
        ("cc", [128, QL], BF16), ("ss", [128, QL], BF16),
    ]:
        d[name] = nc.dram_tensor(name, shape, dt, kind="ExternalInput")
    out_d = nc.dram_tensor("out", [QL, DIM], BF16, kind="ExternalOutput")

    vT_t = d["vT"].rearrange("(a p) n -> a p n", p=128)
    wvT_t = d["wvT"].rearrange("(a p) n -> a p n", p=128)
    woT_t = d["woT"].rearrange("(a p) n -> a p n", p=128)   # [4, 128, DIM]
    out_t = out_d.rearrange("(a p) n -> a p n", p=128)      # [16, 128, DIM]
    q8_t = d["q8"].rearrange("p (a x) -> a p x", a=4)       # [4, 128, 2*QL]
    k8_t = d["k8"].rearrange("p (a x) -> a p x", a=4)
    wq8_t = d["wq8"].rearrange("p (a x) -> a p x", a=4)     # [4, 128, 2*DL]
    wk8_t = d["wk8"].rearrange("p (a x) -> a p x", a=4)

    SWAP_MASK = [(j + 1 if j % 2 == 0 else j - 1) for j in range(32)]

    with TileContext(nc) as tc, ExitStack() as top:
        # DMA priority: the critical-path tensors (k8/wk8, then q8/wq8 and the
        # rotary patterns) go first on their queues; wv/wo/v are needed only
        # once the tick stream is running.
        consts = top.enter_context(tc.tile_pool(name="consts", bufs=1))
        bq_s = consts.tile([128, NPAIR], F32)
        bk_s = consts.tile([128, NPAIR], F32)
        bv_s = consts.tile([1, DL], BF16)
        ones_s = consts.tile([1, 128], BF16)
        cc_s = consts.tile([128, QL], BF16)
        ss_s = consts.tile([128, QL], BF16)
        wo_s = [consts.tile([128, DIM], BF16, tag=f"wo{i}", name=f"wo{i}")
                for i in range(NPAIR)]
        warm = consts.tile([1, 8], F32)

        # ---- persistent activations ----
        qh_pool = top.enter_context(tc.tile_pool(name="qh", bufs=NPAIR))
        kh_pool = top.enter_context(tc.tile_pool(name="kh", bufs=NPAIR))
        qhT = [qh_pool.tile([128, QL], BF16, tag="qh", name=f"qh{i}")
               for i in range(NPAIR)]
        khT = [kh_pool.tile([128, KL], BF16, tag="kh", name=f"kh{i}")
               for i in range(NPAIR)]
        vh_pool = top.enter_context(tc.tile_pool(name="vh", bufs=16))
        vh = [vh_pool.tile([128, NPAIR * 130], BF16, tag="vh", name=f"vh{i}")
              for i in range(16)]
        at_pool = top.enter_context(tc.tile_pool(name="atn", bufs=NPAIR))
        apT = [at_pool.tile([128, QL], BF16, tag="at", name=f"apT{i}")
               for i in range(NPAIR)]

        with ExitStack() as ph:
            # shared pool: fp8 q/k staging first, ets pair-tiles afterwards
            big = ph.enter_context(tc.tile_pool(name="big", bufs=11))
            vstage = ph.enter_context(tc.tile_pool(name="vstage", bufs=8))
            w8_p = ph.enter_context(tc.tile_pool(name="w8p", bufs=8))
            wv_p = ph.enter_context(tc.tile_pool(name="wvp", bufs=8))
            sw_p = ph.enter_context(tc.tile_pool(name="swp", bufs=1))
            t2_p = ph.enter_context(tc.tile_pool(name="t2p", bufs=1))
            atu_p = ph.enter_context(tc.tile_pool(name="atu", bufs=2))
            rc_p = ph.enter_context(tc.tile_pool(name="rcp", bufs=2))
            bt_p = ph.enter_context(tc.tile_pool(name="btp", bufs=2))
            out_p = ph.enter_context(tc.tile_pool(name="outp", bufs=2))
            dscr = ph.enter_context(tc.tile_pool(name="dscr", bufs=8, space="DRAM"))
            psS = ph.enter_context(tc.tile_pool(name="psS", bufs=3, space="PSUM"))
            psPA = ph.enter_context(tc.tile_pool(name="psPA", bufs=1, space="PSUM"))

            # ---- input staging DMAs (critical path first) ----
            ks8 = [big.tile([128, 2, KL], F8, tag="big", name=f"k8_{a}")
                   for a in range(4)]
            for a in range(4):
                nc.sync.dma_start(
                    out=ks8[a],
                    in_=k8_t[a].rearrange("p (i n) -> p i n", i=2))
            wk8 = [w8_p.tile([128, 2, DL], F8, tag="w8", name=f"wk8_{a}")
                   for a in range(4)]
            for a in range(4):
                nc.scalar.dma_start(
                    out=wk8[a],
                    in_=wk8_t[a].rearrange("p (i n) -> p i n", i=2))
            nc.scalar.dma_start(out=bk_s, in_=d["bkp"][:, :])
            nc.scalar.dma_start(out=bq_s, in_=d["bqp"][:, :])
            nc.scalar.dma_start(out=ones_s, in_=d["ones1"][:, :])
            nc.scalar.dma_start(out=cc_s, in_=d["cc"][:, :])
            nc.scalar.dma_start(out=ss_s, in_=d["ss"][:, :])
            nc.scalar.activation(out=warm, in_=ones_s[0:1, 0:8], func=AFT.Exp)
            qs8 = [big.tile([128, 2, QL], F8, tag="big", name=f"q8_{a}")
                   for a in range(4)]
            for a in range(4):
                nc.sync.dma_start(
                    out=qs8[a],
                    in_=q8_t[a].rearrange("p (i n) -> p i n", i=2))
            wq8 = [w8_p.tile([128, 2, DL], F8, tag="w8", name=f"wq8_{a}")
                   for a in range(4)]
            for a in range(4):
                nc.scalar.dma_start(
                    out=wq8[a],
                    in_=wq8_t[a].rearrange("p (i n) -> p i n", i=2))
            vs = [vstage.tile([128, KL], BF16, tag="vst", name=f"vs{a}")
                  for a in range(8)]
            for a in range(8):
                nc.gpsimd.dma_start(out=vs[a], in_=vT_t[a])
            wvs = [wv_p.tile([128, DL], BF16, tag="wv", name=f"wvs{a}")
                   for a in range(8)]
            for a in range(8):
                nc.scalar.dma_start(out=wvs[a], in_=wvT_t[a])
            nc.scalar.dma_start(out=bv_s, in_=d["bv"][:, :])
            for i in range(NPAIR):
                nc.scalar.dma_start(out=wo_s[i], in_=woT_t[i])

            # ---- fp8 DoubleRow K/Q projection + rotary (lead-in) ----
            def rotary(dst, mt):
                for c2 in range(2):
                    cs = slice(c2 * 1024, (c2 + 1) * 1024)
                    qt = dst[mt][:, cs]
                    sw = sw_p.tile([128, 1024], BF16, tag="sw")
                    nc.vector.stream_shuffle(out=sw, in_=qt, mask=SWAP_MASK)
                    nc.vector.tensor_tensor(out=sw, in0=sw, in1=ss_s[:, cs],
                                            op=ALU.mult)
                    t2 = t2_p.tile([128, 1024], BF16, tag="t2")
                    nc.vector.tensor_tensor(out=t2, in0=qt, in1=cc_s[:, cs],
                                            op=ALU.mult)
                    nc.vector.tensor_tensor(out=qt, in0=sw, in1=t2,
                                            op=ALU.add)

            def qkproj(xs8, ws8, b_s, dst, mt):
                """One pair-tile projection: DR chains + bias, then rotary."""
                for c2 in range(2):
                    ps = psS.tile([128, 1024], F32, tag="S", name=f"pj{mt}{c2}")
                    for a in range(4):
                        for n in range(2):
                            nc.tensor.matmul(
                                ps[:, n * 512:(n + 1) * 512],
                                lhsT=ws8[a][:, :, mt * 128:(mt + 1) * 128],
                                rhs=xs8[a][:, :, c2 * 1024 + n * 512:
                                           c2 * 1024 + (n + 1) * 512],
                                start=(a == 0), stop=(a == 3),
                                perf_mode=DRMODE,
                            )
                    nc.vector.tensor_scalar_add(
                        out=dst[mt][:, c2 * 1024:(c2 + 1) * 1024], in0=ps,
                        scalar1=b_s[:, mt:mt + 1])
                rotary(dst, mt)

            qkproj(ks8, wk8, bk_s, khT, 0)
            qkproj(qs8, wq8, bq_s, qhT, 0)
            for mt in range(1, NPAIR):
                qkproj(ks8, wk8, bk_s, khT, mt)
            for mt in range(1, NPAIR):
                qkproj(qs8, wq8, bq_s, qhT, mt)

            # ---- filler queue: V projection first, out-projection later ----
            fillers = deque()

            def pull(n=1):
                for _ in range(n):
                    if fillers:
                        fillers.popleft()()

            def vproj_closure(t):
                def go():
                    ps = psS.tile([128, 512], F32, tag="S", name=f"vp{t}")
                    for a in range(8):
                        nc.tensor.matmul(
                            ps, lhsT=vs[a][:, t * 128:(t + 1) * 128],
                            rhs=wvs[a], start=(a == 0), stop=False)
                    nc.tensor.matmul(ps, lhsT=ones_s, rhs=bv_s,
                                     start=False, stop=True)
                    vtr = vh[t].rearrange("p (g h e) -> p g h e", h=2, e=65)
                    nc.gpsimd.memset(vtr[:, :, :, 64:65], 1.0)
                    psr = ps.rearrange("p (g h e) -> p g h e", h=2, e=64)
                    nc.scalar.copy(out=vtr[:, :, :, 0:64], in_=psr)
                return go

            fillers.extend(vproj_closure(t) for t in range(16))

            def outproj_closures(qts, pool_tags):
                out = []
                state = {}

                def half(qt, dc, pool, tag):
                    def go():
                        if dc == 0:
                            state[qt] = pool.tile([128, 1024], F32, tag=tag,
                                                  name=f"op{qt}")
                        ps = state[qt]
                        for p in range(NPAIR):
                            nc.tensor.matmul(
                                ps[:, dc * 512:(dc + 1) * 512],
                                lhsT=apT[p][:, qt * 128:(qt + 1) * 128],
                                rhs=wo_s[p][:, dc * 512:(dc + 1) * 512],
                                start=(p == 0), stop=(p == NPAIR - 1),
                            )
                        if dc == 1:
                            ot = out_p.tile([128, DIM], BF16, tag="o")
                            nc.vector.tensor_copy(out=ot, in_=ps)
                            nc.sync.dma_start(out=out_t[qt], in_=ot)
                    return go

                for i, qt in enumerate(qts):
                    pool, tag = pool_tags[i % len(pool_tags)]
                    out.append(half(qt, 0, pool, tag))
                    out.append(half(qt, 1, pool, tag))
                return out

            def normalize(pa, u):
                qc, p, hh = u // 8, (u % 8) // 2, u % 2
                atu = atu_p.tile([128, 1024], F32, tag="atu")
                nc.vector.tensor_copy(out=atu[0:65, :], in_=pa[0:65, :])
                ds = dscr.tile([1, 1024], F32, tag="dsc")
                nc.sync.dma_start(out=ds, in_=atu[64:65, :])
                rc8 = rc_p.tile([128, 8], F32, tag="rc8")
                nc.sync.dma_start(out=rc8,
                                  in_=ds.rearrange("a (p e) -> (a p) e", p=128))
                rc8b = rc_p.tile([128, 8], BF16, tag="rc8b")
                with nc.allow_low_precision(
                        reason="softmax denominators; bf16 reciprocal adds "
                               "~0.4% which is within the error budget"):
                    nc.vector.reciprocal(out=rc8b, in_=rc8)
                ds2 = dscr.tile([1, 1024], BF16, tag="ds2")
                nc.sync.dma_start(
                    out=ds2.rearrange("a (p e) -> (a p) e", p=128), in_=rc8b)
                bt = bt_p.tile([64, 1024], BF16, tag="bc")
                nc.sync.dma_start(out=bt,
                                  in_=ds2[0:1, :].to_broadcast([64, 1024]))
                nc.vector.tensor_tensor(
                    out=apT[p][hh * 64:(hh + 1) * 64,
                               qc * 1024:(qc + 1) * 1024],
                    in0=atu[0:64, :], in1=bt[0:64, :], op=ALU.mult)

            # ---- score stream ticks + lagged attn stream ----
            # unit u = (qc, p, h): qc = u//8, p = (u%8)//2, h = u%2
            ets = {}           # tick -> (pair tile, half slice)
            pa_cur = [None]

            def s_tick(t):
                u, mt = divmod(t, 16)
                qc, p, hh = u // 8, (u % 8) // 2, u % 2
                ps = psS.tile([128, 1024], F32, tag="S", name=f"s{t}")
                for n in range(2):
                    nc.tensor.matmul(
                        ps[:, n * 512:(n + 1) * 512],
                        lhsT=khT[p][hh * 64:(hh + 1) * 64,
                                    mt * 128:(mt + 1) * 128],
                        rhs=qhT[p][hh * 64:(hh + 1) * 64,
                                   qc * 1024 + n * 512:
                                   qc * 1024 + (n + 1) * 512],
                        start=True, stop=True,
                        tile_position=(hh * 64, 0),
                    )
                if mt % 2 == 0:
                    pair = big.tile([128, 2048], BF16, tag="big",
                                    name=f"ep{t}")
                    ets[t] = (pair, slice(0, 1024))
                    ets[t + 1] = (pair, slice(1024, 2048))
                e_tile, e_sl = ets[t]
                nc.scalar.activation(out=e_tile[:, e_sl], in_=ps,
                                     func=AFT.Exp, scale=0.125)

            def a_tick(a):
                u, mt = divmod(a, 16)
                qc, p, hh = u // 8, (u % 8) // 2, u % 2
                if mt == 0:
                    pa_cur[0] = psPA.tile([128, 1024], F32, tag="PA",
                                          name=f"pa{u}")
                pa = pa_cur[0]
                e_tile, e_sl = ets.pop(a)
                lhs = vh[mt][:, p * 130 + hh * 65: p * 130 + (hh + 1) * 65]
                base = e_sl.start
                for n in range(2):
                    nc.tensor.matmul(
                        pa[0:65, n * 512:(n + 1) * 512],
                        lhsT=lhs,
                        rhs=e_tile[:, base + n * 512: base + (n + 1) * 512],
                        start=(mt == 0), stop=(mt == 15),
                    )
                if mt == 15:
                    normalize(pa, u)
                    if u == 7:            # qc=0 attn complete
                        fillers.extend(
                            outproj_closures(range(8), [(psPA, "PA")]))

            for t in range(256):
                if t < 16:
                    pull(1)           # V projection rides the early ticks
                s_tick(t)
                if t >= LAG:
                    a = t - LAG
                    a_tick(a)
                    if a % 16 == 15 and a >= 128:
                        pull(2)       # qc1 boundary: one out-projection chain
            for a in range(256 - LAG, 256):
                a_tick(a)
                pull(1)
            while fillers:
                pull(1)
            # tail: alternate PSUM pools so chains overlap their copy-out
            for fn in outproj_closures(range(8, 16),
                                       [(psPA, "PA"), (psS, "S")]):
                fn()

    nc.compile()
    _NC_CACHE["nc"] = nc
    return nc


def _pack_fp8_pairs(mat_T, scale):
    """[1024, C] f32 -> [128, 4, 2, C] fp8 with x[p, a2, i, c] = row (a2*2+i)*128+p."""
    C = mat_T.shape[1]
    x = np.clip(mat_T * scale, -240, 240).astype(f8e4)
    x = x.reshape(4, 2, 128, C).transpose(2, 0, 1, 3)     # [128, 4, 2, C]
    return np.ascontiguousarray(x.reshape(128, 8 * C))


def _make_in_maps(q, k, v, Wq, bq, Wk, bk, Wv, bv, Wo, bo):
    q, k, v = (np.asarray(x, np.float32) for x in (q, k, v))
    Wq, Wk, Wv, Wo = (np.asarray(x, np.float32) for x in (Wq, Wk, Wv, Wo))
    bq, bk, bv, bo = (np.asarray(x, np.float32) for x in (bq, bk, bv, bo))
    cc, ss = _rot_patterns()
    ones1 = np.ones((1, 128), np.float32)
    in_maps = []
    for c in range(NCORE):
        b, g = divmod(c, G)
        gs = slice(g * DL, (g + 1) * DL)
        in_maps.append({
            "q8": _pack_fp8_pairs(np.ascontiguousarray(q[b].T), SCI),
            "k8": _pack_fp8_pairs(np.ascontiguousarray(k[b].T), SCI),
            "wq8": _pack_fp8_pairs(np.ascontiguousarray(Wq[gs, :].T), SCW),
            "wk8": _pack_fp8_pairs(np.ascontiguousarray(Wk[gs, :].T), SCW),
            "vT": np.ascontiguousarray(v[b].T).astype(bf16),
            "wvT": np.ascontiguousarray(Wv[gs, :].T).astype(bf16),
            "woT": np.ascontiguousarray(Wo[:, gs].T).astype(bf16),
            # biases pre-scaled by 128 (descale folded into cc/ss)
            "bqp": np.ascontiguousarray(
                (bq[gs] / DESC).reshape(NPAIR, 128).T),
            "bkp": np.ascontiguousarray(
                (bk[gs] / DESC).reshape(NPAIR, 128).T),
            "bv": np.ascontiguousarray(bv[gs][None, :]).astype(bf16),
            "ones1": ones1.astype(bf16),
            "cc": cc.astype(bf16), "ss": ss.astype(bf16),
        })
    return in_maps


def run(inputs: dict, trace: bool = False, tmpdir: str | None = None):
    """Returns (out [B, QL, DIM] f32, exec_time_ns or None)."""
    from concourse.bass_utils import run_bass_kernel_spmd

    nc = _build_nc()
    in_maps = _make_in_maps(**inputs)
    res = run_bass_kernel_spmd(nc, in_maps, list(range(NCORE)), trace=trace,
                               tmpdir=tmpdir)
    globals()["LAST_RES"] = res
    bo = np.asarray(inputs["bo"], np.float32)
    outs = [np.asarray(res.results[i]["out"], np.float32) for i in range(NCORE)]
    out = np.stack([outs[G * b] + outs[G * b + 1] for b in range(B)])
    out += bo[None, None, :]
    return out.astype(np.float32), res.exec_time_ns


def kernel(**inputs) -> np.ndarray:
    out, _ = run(inputs, trace=False)
    return out
